# revision 45
# baseline (speedup 1.0000x reference)
"""Trainium2 Bass kernel for causal multi-head attention (dense transformer block).

Math (reference semantics):
    qkv = x @ w_qkv.T ; split into Q,K,V heads [B,H,T,dk]
    (rotary in the reference rotates Q and K of head h by a constant,
     time-independent orthogonal rotation R_h; since scores = (R_h q)·(R_h k)
     = q·k, the rotation cancels exactly and is skipped here)
    scores = causal_mask(Q @ K.T / sqrt(dk)); attn = softmax(scores)
    out = attn @ V ; y = out @ w_o.T
Sharding: head-parallel over 8 cores (2 heads/core, both batches). Each core
computes a partial y (its heads' contribution through w_o columns); the host
sums the 8 partials (the "all-reduce").

Design (everything fp16 on-chip, fp32 PSUM accumulation; host pre-packs
x/w/wo into layouts that make every load a single contiguous DMA):
  phase 1 per (b, nb): one batched x DMA; Q,K as [dk, tok] via m-major
    accumulation; V computed DIRECTLY in [tok, dk] layout (x-chunk stationary)
    so no PE transposes are needed. QT/KT/V double-buffered across batches.
  phase 2 per (b, qb): causal-exact diagonal tiles with shrinking free size
    (full PE rate at any ap since fp16); softmax denominator accumulated on
    DVE in fp16 (acc += ex per key-tile), summed across partitions on the
    otherwise-idle Pool engine (partition_all_reduce), reciprocal on DVE over
    all 128 rows (DVE cost depends on free size only) — softmax costs the PE
    nothing and takes no PSUM banks.
  schedule: generator round-robin interleaves, per query block qb, the two
    heads' attention, the deferred output projection of block qb-1, and
    phase 1 of block nb=qb+1 (or the next batch's first block) — the PE queue
    always holds ready work across exp/reciprocal latencies.
  phase 3: wide [128, 2048] y stores, one DMA per token tile (per-eb DMAs on
    the last tile to shorten the final drain).
"""

import contextlib

import numpy as np

import concourse.bacc as bacc
import concourse.bass as bass
import concourse.bass_isa as bass_isa
import concourse.mybir as mybir
import concourse.tile as tile
from concourse import bass_utils

B, T, D, H, DK = 2, 2048, 2048, 16, 128
NCORES = 8
HPC = H // NCORES  # heads per core
P = 128
NB = 512           # token/query block
KC = D // P        # 16 contraction chunks
QB = T // NB       # 4 query blocks per batch
NT = T // P        # 16 token tiles per batch
FP32 = mybir.dt.float32
FP16 = mybir.dt.float16
SCALE = 1.0 / np.sqrt(DK)

DEFAULT_OPTS = dict(
    skew=2, ex_bufs=8, pss_bufs=4, po_bufs=2, py_bufs=2, yt_bufs=4,
    acc_bufs=2, xt_bufs=2, proj_delay=2, bc_direct=False, warm_chunks=4,
    rr_ph1=True, warm=True, warm_v_steady=True, pool_copy=False,
    bc_eng="vector", denom_split=False, tail_rotate=True, fine_warm=True,
    pool_denom=True, tail_dma="lasttt", ph1_copy="sv", proj_copy="sv",
    mask_eng="vector", late_wo=True, ilv_ph1=True, warm_xring="sync",
    proj_delay_tail=6,
)


def build(debug=False, **opts):
    o = dict(DEFAULT_OPTS); o.update(opts)
    nc = bacc.Bacc("TRN2", target_bir_lowering=False, debug=False,
                   num_devices=NCORES)
    xr = nc.dram_tensor("xr", [P, B * QB * KC * NB], FP16, kind="ExternalInput")
    wq = nc.dram_tensor("wq", [P, KC * 6 * DK], FP16, kind="ExternalInput")
    wo = nc.dram_tensor("wo", [P, HPC * D], FP16, kind="ExternalInput")
    maskt = nc.dram_tensor("maskt", [P, P], FP16, kind="ExternalInput")
    y = nc.dram_tensor("y", [B * T, D], FP16, kind="ExternalOutput")
    dbg = {}
    if debug:
        for nm in ("QT0", "KT0", "V0", "outT0"):
            dbg[nm] = nc.dram_tensor("dbg_" + nm, [P, B * T], FP16,
                                     kind="ExternalOutput")

    with tile.TileContext(nc) as tc:
        with (
            tc.tile_pool(name="const", bufs=1) as cpool,
            tc.tile_pool(name="xp", bufs=2) as xpool,
            tc.tile_pool(name="qkv", bufs=1) as qpool,
            tc.tile_pool(name="attn", bufs=1) as apool,
            tc.tile_pool(name="ps", bufs=1, space="PSUM") as pspool,
        ):
            wsb = cpool.tile([P, KC * 6 * DK], FP16, name="wsb")
            wosb = cpool.tile([P, HPC * D], FP16, name="wosb")
            maskd = cpool.tile([P, P], FP16, name="maskd")
            onc = cpool.tile([P, 1], FP16, name="onc")
            onr = cpool.tile([1, P], FP16, name="onr")
            nc.vector.memset(onc[:], 1.0)
            nc.vector.memset(onr[:], 1.0)

            # per-batch per-head tensors: QT/KT/V double-buffered across
            # batches so next-batch phase 1 can interleave with the current
            # batch's last attention block without clobbering its K/V reads.
            QTb = [[qpool.tile([P, T], FP16, name=f"QT{h}_{b}") for h in range(HPC)]
                   for b in range(B)]
            KTb = [[qpool.tile([P, T], FP16, name=f"KT{h}_{b}") for h in range(HPC)]
                   for b in range(B)]
            Vb = [[qpool.tile([P, T], FP16, name=f"V{h}_{b}") for h in range(HPC)]
                  for b in range(B)]
            # outT is safe single-buffered: proj(b,qb) only reads its own
            # qb column block, disjoint from the next batch's writes
            outT = [qpool.tile([P, T], FP16, name=f"oT{h}") for h in range(HPC)]

            # lhsT slices for phase 1: (m, k) -> weight chunk [128 d, 128 dk]
            # column order per k-chunk: Qh0 Kh0 Qh1 Kh1 Vh0 Vh1
            def wslice(m, k):
                c0 = k * 6 * DK + m * DK
                return wsb[:, c0:c0 + DK]

            copy_flip = [0]
            engsets = {
                "sv": [nc.scalar, nc.vector],
                "v": [nc.vector],
                "s": [nc.scalar],
                "vp": [nc.vector, nc.gpsimd],
                "svp": [nc.scalar, nc.vector, nc.gpsimd],
                "sp": [nc.scalar, nc.gpsimd],
                "p": [nc.gpsimd],
            }

            def psum_copy(dst, src, which="sv"):
                # rotate PSUM->SBUF drains across the engine set
                engines = engsets[which]
                eng = engines[copy_flip[0] % len(engines)]
                if eng is nc.scalar:
                    eng.copy(dst, src)
                else:
                    eng.tensor_copy(dst, src)
                copy_flip[0] += 1

            # ---------------- phase 1 ----------------
            def ph1_gen(b, nb, warm=False):
                xt = xpool.tile([P, KC * NB], FP16, name=f"x_{b}_{nb}",
                                tag="xt", bufs=o["xt_bufs"])
                col0 = (b * QB + nb) * KC * NB
                if warm:
                    # chunked loads interleaved with weight chunks so the PE
                    # can start after the first x+w chunk pair; the first few
                    # k-chunks load individually to minimize time-to-first-mm
                    if o["fine_warm"]:
                        kranges = [(0, 1), (1, 2), (2, 4), (4, 6), (6, 8),
                                   (8, 10), (10, 12), (12, 14), (14, 16)]
                    else:
                        kranges = [(4 * c, 4 * c + 4) for c in range(4)]
                    xring = nc.scalar if o["warm_xring"] in ("vector", "scalar") else nc.sync
                    for k0, k1 in kranges:
                        nc.sync.dma_start(wsb[:, k0 * 6 * DK:k1 * 6 * DK],
                                          wq[:, k0 * 6 * DK:k1 * 6 * DK])
                        xring.dma_start(xt[:, k0 * NB:k1 * NB],
                                        xr[:, col0 + k0 * NB:col0 + k1 * NB])
                    if not o["late_wo"]:
                        nc.sync.dma_start(maskd[:], maskt[:, :])
                        nc.sync.dma_start(wosb[:], wo[:, :])
                else:
                    nc.sync.dma_start(xt[:], xr[:, col0:col0 + KC * NB])
                    if o["late_wo"] and b == 0 and nb == 1:
                        nc.sync.dma_start(maskd[:], maskt[:, :])
                        nc.sync.dma_start(wosb[:], wo[:, :])
                tsl = slice(nb * NB, (nb + 1) * NB)
                QT, KT, V = QTb[b], KTb[b], Vb[b]
                mdest = [QT[0], KT[0], QT[1], KT[1]]
                if warm:
                    # chunk-major: all 6 PSUM groups live, consume x chunks
                    # as they arrive
                    psm = [pspool.tile([P, NB], FP32, name=f"p1w_{m}",
                                       tag=("ps_s" if m < 3 else "ps_y"),
                                       bufs=(o["pss_bufs"] if m < 3 else o["py_bufs"]))
                           for m in range(4)]
                    psv = [pspool.tile([P, NB], FP32, name=f"p1wv_{h}",
                                       tag="ps_o", bufs=o["po_bufs"])
                           for h in range(HPC)]
                    for k0, k1 in kranges:
                        for m in range(4):
                            for k in range(k0, k1):
                                nc.tensor.matmul(psm[m][:], wslice(m, k),
                                                 xt[:, k * NB:(k + 1) * NB],
                                                 start=(k == 0), stop=(k == KC - 1))
                        if not o["warm_v_steady"]:
                            for h in range(HPC):
                                for q in range(4):
                                    qs = slice(q * P, (q + 1) * P)
                                    for k in range(k0, k1):
                                        nc.tensor.matmul(
                                            psv[h][:, qs],
                                            xt[:, k * NB + q * P:k * NB + (q + 1) * P],
                                            wslice(4 + h, k),
                                            start=(k == 0), stop=(k == KC - 1),
                                            skip_group_check=True)
                        yield
                    for m in range(4):
                        psum_copy(mdest[m][:, tsl], psm[m][:], o["ph1_copy"])
                    if o["warm_v_steady"]:
                        for h in range(HPC):
                            for q in range(4):
                                qs = slice(q * P, (q + 1) * P)
                                for k in range(KC):
                                    nc.tensor.matmul(
                                        psv[h][:, qs],
                                        xt[:, k * NB + q * P:k * NB + (q + 1) * P],
                                        wslice(4 + h, k),
                                        start=(k == 0), stop=(k == KC - 1),
                                        skip_group_check=True)
                            yield
                    for h in range(HPC):
                        psum_copy(V[h][:, tsl], psv[h][:], o["ph1_copy"])
                    yield
                else:
                    for m in range(4):
                        ps = pspool.tile([P, NB], FP32, name=f"p1_{b}_{nb}_{m}",
                                         tag="ps_s", bufs=o["pss_bufs"])
                        for k in range(KC):
                            nc.tensor.matmul(ps[:], wslice(m, k),
                                             xt[:, k * NB:(k + 1) * NB],
                                             start=(k == 0), stop=(k == KC - 1))
                        psum_copy(mdest[m][:, tsl], ps[:], o["ph1_copy"])
                        yield
                    for h in range(HPC):
                        psv = pspool.tile([P, NB], FP32, name=f"p1v_{b}_{nb}_{h}",
                                          tag="ps_o", bufs=o["po_bufs"])
                        for q in range(4):
                            qs = slice(q * P, (q + 1) * P)
                            for k in range(KC):
                                nc.tensor.matmul(
                                    psv[:, qs],
                                    xt[:, k * NB + q * P:k * NB + (q + 1) * P],
                                    wslice(4 + h, k),
                                    start=(k == 0), stop=(k == KC - 1),
                                    skip_group_check=True)
                        psum_copy(V[h][:, tsl], psv[:], o["ph1_copy"])
                        yield

            # ---------------- phase 2: attention ----------------
            def attn_gen(h, b, qb):
                QT, KT, V = QTb[b], KTb[b], Vb[b]
                # key tiles: 4*qb full-width off-diagonal, then 4 diagonal
                # tiles with shrinking query range
                tiles = [(kt, NB, 0, False) for kt in range(4 * qb)]
                tiles += [(4 * qb + j, NB - j * P, j * P, True) for j in range(4)]
                n = len(tiles)
                q0 = qb * NB
                ps_o = pspool.tile([P, NB], FP32, name=f"po_{b}_{h}_{qb}",
                                   tag="ps_o", bufs=o["po_bufs"])
                acc = apool.tile([P, NB], FP16, name=f"acc_{b}_{h}_{qb}",
                                 tag=f"acc{h}", bufs=o["acc_bufs"])
                pss = {}

                def issue_scores(i):
                    kt, w, qo, diag = tiles[i]
                    ps = pspool.tile([P, NB], FP32, name=f"pss_{b}_{h}_{qb}_{kt}",
                                     tag="ps_s", bufs=o["pss_bufs"])
                    ksl = slice(kt * P, (kt + 1) * P)
                    nc.tensor.matmul(ps[:, 0:w], KT[h][:, ksl],
                                     QT[h][:, q0 + qo:q0 + qo + w],
                                     start=True, stop=True)
                    pss[i] = ps

                for i in range(min(o["skew"], n)):
                    issue_scores(i)
                for i in range(n):
                    kt, w, qo, diag = tiles[i]
                    ps_s = pss.pop(i)
                    ex = apool.tile([P, NB], FP16, name=f"ex_{b}_{h}_{qb}_{kt}",
                                    tag="ex", bufs=o["ex_bufs"])
                    nc.scalar.activation(ex[:, 0:w], ps_s[:, 0:w],
                                         mybir.ActivationFunctionType.Exp,
                                         scale=SCALE)
                    if diag:
                        meng = nc.gpsimd if o["mask_eng"] == "gpsimd" else nc.vector
                        meng.tensor_mul(ex[:, 0:P], ex[:, 0:P], maskd[:])
                    if i + o["skew"] < n:
                        issue_scores(i + o["skew"])
                    ksl = slice(kt * P, (kt + 1) * P)
                    nc.tensor.matmul(ps_o[:, qo:NB], V[h][:, ksl], ex[:, 0:w],
                                     start=(i == 0), stop=(i == n - 1),
                                     skip_group_check=True)
                    if i == 0:
                        nc.vector.tensor_copy(acc[:, 0:w], ex[:, 0:w])
                    else:
                        nc.vector.tensor_add(acc[:, qo:NB], acc[:, qo:NB],
                                             ex[:, 0:w])
                    yield
                # softmax denominator + normalize
                osl = slice(q0, q0 + NB)
                if o["pool_denom"]:
                    # all-partition sum on the idle Pool engine; reciprocal on
                    # DVE costs by free size only, so take it on all 128 rows
                    dn = apool.tile([P, NB], FP32, name=f"dn_{b}_{h}_{qb}",
                                    tag=f"dn{h}", bufs=2)
                    nc.gpsimd.partition_all_reduce(dn[:], acc[:], channels=P,
                                                   reduce_op=bass_isa.ReduceOp.add)
                    bc = apool.tile([P, NB], FP32, name=f"bc_{b}_{h}_{qb}",
                                    tag=f"bc{h}", bufs=2)
                    with nc.allow_low_precision(reason="fp32 softmax recip"):
                        nc.vector.reciprocal(bc[:], dn[:])
                    yield
                    nc.vector.tensor_mul(outT[h][:, osl], ps_o[:], bc[:])
                    yield
                    return
                ps_d = pspool.tile([1, NB], FP32, name=f"pd_{b}_{h}_{qb}",
                                   tag="ps_y", bufs=o["py_bufs"])
                nc.tensor.matmul(ps_d[:1, :], onc[:, :], acc[:],
                                 start=True, stop=True)
                rec = apool.tile([1, NB], FP16, name=f"rec_{b}_{h}_{qb}",
                                 tag=f"rec{h}", bufs=2)
                with nc.allow_low_precision(reason="fp16 softmax recip"):
                    nc.vector.reciprocal(rec[:1, :], ps_d[:1, :])
                yield
                ps_b = pspool.tile([P, NB], FP32, name=f"pb_{b}_{h}_{qb}",
                                   tag="ps_y", bufs=o["py_bufs"])
                nc.tensor.matmul(ps_b[:], onr[:1, :], rec[:1, :],
                                 start=True, stop=True)
                bc = apool.tile([P, NB], FP32, name=f"bc_{b}_{h}_{qb}",
                                tag="bc", bufs=2)
                if o["bc_eng"] == "gpsimd":
                    nc.gpsimd.tensor_copy(bc[:], ps_b[:])
                else:
                    nc.vector.tensor_copy(bc[:], ps_b[:])
                nc.vector.tensor_mul(outT[h][:, osl], ps_o[:], bc[:])
                yield

            # ---------------- phase 3: output projection ----------------
            def proj_gen(b, qb, tail=False):
                last_tt = 4 * qb + 3
                eb_tts = {"eb": set(range(4 * qb, 4 * qb + 4)),
                          "lasttt": {last_tt},
                          "last2": {last_tt - 1, last_tt},
                          "tt": set()}[o["tail_dma"]] if tail else set()
                tags = ([("ps_y", o["py_bufs"]), ("ps_s", o["pss_bufs"]),
                         ("ps_o", o["po_bufs"])] if tail and o["tail_rotate"]
                        else [("ps_y", o["py_bufs"])])
                ti = 0
                for tt in range(4 * qb, 4 * qb + 4):
                    yt = apool.tile([P, D], FP16, name=f"yt_{b}_{tt}",
                                    tag="yt", bufs=o["yt_bufs"])
                    csl = slice(tt * P, (tt + 1) * P)
                    for eb in range(4):
                        tag, tb = tags[ti % len(tags)]
                        ti += 1
                        ps = pspool.tile([P, NB], FP32, name=f"py_{b}_{tt}_{eb}",
                                         tag=tag, bufs=tb)
                        for h in range(HPC):
                            nc.tensor.matmul(ps[:], outT[h][:, csl],
                                             wosb[:, h * D + eb * NB:h * D + (eb + 1) * NB],
                                             start=(h == 0), stop=(h == HPC - 1))
                        psum_copy(yt[:, eb * NB:(eb + 1) * NB], ps[:],
                                  "sv" if tail else o["proj_copy"])
                        if tt in eb_tts:
                            nc.sync.dma_start(
                                y[b * T + tt * P:b * T + (tt + 1) * P,
                                  eb * NB:(eb + 1) * NB],
                                yt[:, eb * NB:(eb + 1) * NB])
                        yield
                    if tt not in eb_tts:
                        nc.sync.dma_start(y[b * T + tt * P:b * T + (tt + 1) * P, :],
                                          yt[:])
                    yield

            def rr(gens, delays=None):
                delays = delays or [0] * len(gens)
                done = [False] * len(gens)
                rounds = 0
                while not all(done):
                    for gi, g in enumerate(gens):
                        if done[gi] or rounds < delays[gi]:
                            continue
                        try:
                            next(g)
                        except StopIteration:
                            done[gi] = True
                    rounds += 1

            # ---------------- driver ----------------
            pending_proj = None  # proj generator deferred across qb/batch
            if o["ilv_ph1"]:
                # software-pipeline: ph1 of block nb+1 (or next batch's nb0)
                # interleaves with attention of block qb=nb
                rr([ph1_gen(0, 0, warm=o["warm"])])
                for b in range(B):
                    for qb in range(QB):
                        gens = []
                        delays = []
                        if qb < QB - 1:
                            gens.append(ph1_gen(b, qb + 1))
                            delays.append(0)
                        elif b + 1 < B:
                            gens.append(ph1_gen(b + 1, 0))
                            delays.append(0)
                        gens += [attn_gen(0, b, qb), attn_gen(1, b, qb)]
                        delays += [0, 0]
                        if pending_proj is not None:
                            gens.append(pending_proj)
                            delays.append(o["proj_delay_tail"]
                                          if (b == B - 1 and qb == QB - 1)
                                          else o["proj_delay"])
                        rr(gens, delays)
                        pending_proj = proj_gen(b, qb,
                                                tail=(b == B - 1 and qb == QB - 1))
                    if debug:
                        for nm, t in (("QT0", QTb[b][0]), ("KT0", KTb[b][0]),
                                      ("V0", Vb[b][0]), ("outT0", outT[0])):
                            nc.sync.dma_start(dbg[nm][:, b * T:(b + 1) * T], t[:])
                rr([pending_proj])
            else:
                for b in range(B):
                    for nb in range(QB):
                        g = ph1_gen(b, nb, warm=(o["warm"] and b == 0 and nb == 0))
                        if pending_proj is not None and o["rr_ph1"]:
                            rr([g, pending_proj])
                            pending_proj = None
                        else:
                            rr([g])
                    if pending_proj is not None:
                        rr([pending_proj])
                        pending_proj = None
                    for qb in range(QB):
                        gens = [attn_gen(0, b, qb), attn_gen(1, b, qb)]
                        delays = [0, 0]
                        if pending_proj is not None:
                            gens.append(pending_proj)
                            delays.append(o["proj_delay"])
                        rr(gens, delays)
                        pending_proj = proj_gen(b, qb, tail=(b == B - 1 and qb == QB - 1))
                    if debug:
                        for nm, t in (("QT0", QTb[b][0]), ("KT0", KTb[b][0]),
                                      ("V0", Vb[b][0]), ("outT0", outT[0])):
                            nc.sync.dma_start(dbg[nm][:, b * T:(b + 1) * T], t[:])
                rr([pending_proj])

    nc.compile()
    return nc


def prep_inputs(x, w_qkv, w_o):
    """Host-side shard prep. Returns per-core input maps."""
    x = np.asarray(x, np.float32).reshape(B, T, D)
    # xr[p, (b, nb, k, q)] = x[b, nb*NB+q, k*P+p]
    xr = x.reshape(B, QB, NB, KC, P).transpose(4, 0, 1, 3, 2)
    xr = np.ascontiguousarray(xr.reshape(P, B * QB * KC * NB)).astype(np.float16)
    w_qkv = np.asarray(w_qkv, np.float32)
    w_o = np.asarray(w_o, np.float32)

    mask = (np.arange(P)[:, None] <= np.arange(P)[None, :]).astype(np.float16)

    in_maps = []
    for c in range(NCORES):
        h0, h1 = HPC * c, HPC * c + 1
        blocks = [w_qkv[h0 * DK:(h0 + 1) * DK],               # Q h0
                  w_qkv[D + h0 * DK:D + (h0 + 1) * DK],       # K h0
                  w_qkv[h1 * DK:(h1 + 1) * DK],               # Q h1
                  w_qkv[D + h1 * DK:D + (h1 + 1) * DK],       # K h1
                  w_qkv[2 * D + h0 * DK:2 * D + (h0 + 1) * DK],  # V h0
                  w_qkv[2 * D + h1 * DK:2 * D + (h1 + 1) * DK]]  # V h1
        wstack = np.stack(blocks)                   # [6, DK(j), D(d)]
        t = wstack.reshape(6, DK, KC, P)            # (m, j, k, p)
        wq_host = np.ascontiguousarray(
            t.transpose(3, 2, 0, 1).reshape(P, KC * 6 * DK)).astype(np.float16)
        wof = w_o[:, c * HPC * DK:(c + 1) * HPC * DK]  # [D(e), 256(f)]
        wo_host = np.ascontiguousarray(
            wof.T.reshape(HPC, P, D).transpose(1, 0, 2).reshape(P, HPC * D)
        ).astype(np.float16)
        in_maps.append({"xr": xr, "wq": wq_host, "wo": wo_host, "maskt": mask})
    return in_maps


_nc_cache = {}


def get_nc(debug=False, **opts):
    key = (debug, tuple(sorted(opts.items())))
    if key not in _nc_cache:
        _nc_cache[key] = build(debug=debug, **opts)
    return _nc_cache[key]


def run(x, w_qkv, w_o, debug=False, **opts):
    nc = get_nc(debug=debug, **opts)
    in_maps = prep_inputs(x, w_qkv, w_o)
    res = bass_utils.run_bass_kernel_spmd(nc, in_maps, core_ids=list(range(NCORES)))
    return res


def kernel(x, w_qkv, w_o):
    res = run(x, w_qkv, w_o)
    y = res.results[0]["y"].astype(np.float64)
    for c in range(1, NCORES):
        y += res.results[c]["y"]
    return y.astype(np.float32).reshape(B, T, D)



# revision 49
# speedup vs baseline: 1.0007x; 1.0007x over previous
"""Trainium2 Bass kernel for causal multi-head attention (dense transformer block).

Math (reference semantics):
    qkv = x @ w_qkv.T ; split into Q,K,V heads [B,H,T,dk]
    (rotary in the reference rotates Q and K of head h by a constant,
     time-independent orthogonal rotation R_h; since scores = (R_h q)·(R_h k)
     = q·k, the rotation cancels exactly and is skipped here)
    scores = causal_mask(Q @ K.T / sqrt(dk)); attn = softmax(scores)
    out = attn @ V ; y = out @ w_o.T
Sharding: head-parallel over 8 cores (2 heads/core, both batches). Each core
computes a partial y (its heads' contribution through w_o columns); the host
sums the 8 partials (the "all-reduce").

Design (everything fp16 on-chip, fp32 PSUM accumulation; host pre-packs
x/w/wo into layouts that make every load a single contiguous DMA):
  phase 1 per (b, nb): one batched x DMA; Q,K as [dk, tok] via m-major
    accumulation; V computed DIRECTLY in [tok, dk] layout (x-chunk stationary)
    so no PE transposes are needed. QT/KT/V double-buffered across batches.
  phase 2 per (b, qb): causal-exact diagonal tiles with shrinking free size
    (full PE rate at any ap since fp16); softmax denominator accumulated on
    DVE in fp16 (acc += ex per key-tile), summed across partitions on the
    otherwise-idle Pool engine (partition_all_reduce), reciprocal on DVE over
    all 128 rows (DVE cost depends on free size only) — softmax costs the PE
    nothing and takes no PSUM banks.
  schedule: generator round-robin interleaves, per query block qb, the two
    heads' attention, the deferred output projection of block qb-1, and
    phase 1 of block nb=qb+1 (or the next batch's first block) — the PE queue
    always holds ready work across exp/reciprocal latencies.
  phase 3: wide [128, 2048] y stores, one DMA per token tile (per-eb DMAs on
    the last tile to shorten the final drain).
"""

import contextlib

import numpy as np

import concourse.bacc as bacc
import concourse.bass as bass
import concourse.bass_isa as bass_isa
import concourse.mybir as mybir
import concourse.tile as tile
from concourse import bass_utils

B, T, D, H, DK = 2, 2048, 2048, 16, 128
NCORES = 8
HPC = H // NCORES  # heads per core
P = 128
NB = 512           # token/query block
KC = D // P        # 16 contraction chunks
QB = T // NB       # 4 query blocks per batch
NT = T // P        # 16 token tiles per batch
FP32 = mybir.dt.float32
FP16 = mybir.dt.float16
SCALE = 1.0 / np.sqrt(DK)

DEFAULT_OPTS = dict(
    skew=2, ex_bufs=8, pss_bufs=4, po_bufs=2, py_bufs=2, yt_bufs=4,
    acc_bufs=2, xt_bufs=2, proj_delay=2, bc_direct=False, warm_chunks=4,
    rr_ph1=True, warm=True, warm_v_steady=True, pool_copy=False,
    bc_eng="vector", denom_split=False, tail_rotate=True, fine_warm=True,
    pool_denom=True, tail_dma="lasttt", ph1_copy="sv", proj_copy="sv",
    mask_eng="vector", late_wo=True, ilv_ph1=True, warm_xring="sync",
    proj_delay_tail=6, fillers=0, kr_fillers="", attn_first=False, ph1_delay=0,
)


def build(debug=False, **opts):
    o = dict(DEFAULT_OPTS); o.update(opts)
    nc = bacc.Bacc("TRN2", target_bir_lowering=False, debug=False,
                   num_devices=NCORES)
    xr = nc.dram_tensor("xr", [P, B * QB * KC * NB], FP16, kind="ExternalInput")
    wq = nc.dram_tensor("wq", [P, KC * 6 * DK], FP16, kind="ExternalInput")
    wo = nc.dram_tensor("wo", [P, HPC * D], FP16, kind="ExternalInput")
    maskt = nc.dram_tensor("maskt", [P, P], FP16, kind="ExternalInput")
    y = nc.dram_tensor("y", [B * T, D], FP16, kind="ExternalOutput")
    dbg = {}
    if debug:
        for nm in ("QT0", "KT0", "V0", "outT0"):
            dbg[nm] = nc.dram_tensor("dbg_" + nm, [P, B * T], FP16,
                                     kind="ExternalOutput")

    with tile.TileContext(nc) as tc:
        with (
            tc.tile_pool(name="const", bufs=1) as cpool,
            tc.tile_pool(name="xp", bufs=2) as xpool,
            tc.tile_pool(name="qkv", bufs=1) as qpool,
            tc.tile_pool(name="attn", bufs=1) as apool,
            tc.tile_pool(name="ps", bufs=1, space="PSUM") as pspool,
        ):
            wsb = cpool.tile([P, KC * 6 * DK], FP16, name="wsb")
            wosb = cpool.tile([P, HPC * D], FP16, name="wosb")
            maskd = cpool.tile([P, P], FP16, name="maskd")
            onc = cpool.tile([P, 1], FP16, name="onc")
            onr = cpool.tile([1, P], FP16, name="onr")
            nc.vector.memset(onc[:], 1.0)
            nc.vector.memset(onr[:], 1.0)

            # per-batch per-head tensors: QT/KT/V double-buffered across
            # batches so next-batch phase 1 can interleave with the current
            # batch's last attention block without clobbering its K/V reads.
            QTb = [[qpool.tile([P, T], FP16, name=f"QT{h}_{b}") for h in range(HPC)]
                   for b in range(B)]
            KTb = [[qpool.tile([P, T], FP16, name=f"KT{h}_{b}") for h in range(HPC)]
                   for b in range(B)]
            Vb = [[qpool.tile([P, T], FP16, name=f"V{h}_{b}") for h in range(HPC)]
                  for b in range(B)]
            # outT is safe single-buffered: proj(b,qb) only reads its own
            # qb column block, disjoint from the next batch's writes
            outT = [qpool.tile([P, T], FP16, name=f"oT{h}") for h in range(HPC)]

            # lhsT slices for phase 1: (m, k) -> weight chunk [128 d, 128 dk]
            # layout: QK region first (k-major, Qh0 Kh0 Qh1 Kh1 per k), then
            # V region (k-major, Vh0 Vh1 per k) so warm-start QK loads are
            # small and V weights arrive in one deferred DMA
            QKW = KC * 4 * DK  # 8192: start of the V region

            def wslice(m, k):
                if m < 4:
                    return wsb[:, k * 4 * DK + m * DK:k * 4 * DK + (m + 1) * DK]
                c0 = QKW + k * 2 * DK + (m - 4) * DK
                return wsb[:, c0:c0 + DK]

            fill_state = {"tile": None, "dum": None, "n": 0}

            def pe_filler(count):
                # dependency-free matmuls that keep the PE p-state ramped
                # while real work is DMA-gated (output never read)
                if count <= 0:
                    return
                if fill_state["dum"] is None:
                    dum = cpool.tile([P, NB], FP16, name="fill_dum")
                    nc.vector.memset(dum[:], 0.0)
                    fill_state["dum"] = dum
                    fill_state["tile"] = pspool.tile([1, NB], FP32, name="fill_ps",
                                                     tag="ps_y", bufs=o["py_bufs"])
                for _ in range(count):
                    nc.tensor.matmul(fill_state["tile"][:1, :], onc[:, :],
                                     fill_state["dum"][:], start=True, stop=True,
                                     skip_group_check=True)
                    fill_state["n"] += 1

            copy_flip = [0]
            engsets = {
                "sv": [nc.scalar, nc.vector],
                "v": [nc.vector],
                "s": [nc.scalar],
                "vp": [nc.vector, nc.gpsimd],
                "svp": [nc.scalar, nc.vector, nc.gpsimd],
                "sp": [nc.scalar, nc.gpsimd],
                "p": [nc.gpsimd],
            }

            def psum_copy(dst, src, which="sv"):
                # rotate PSUM->SBUF drains across the engine set
                engines = engsets[which]
                eng = engines[copy_flip[0] % len(engines)]
                if eng is nc.scalar:
                    eng.copy(dst, src)
                else:
                    eng.tensor_copy(dst, src)
                copy_flip[0] += 1

            # ---------------- phase 1 ----------------
            xt_cache = {}

            def get_xt(b, nb, defer_dma=False):
                # tile + (unless deferred) its load DMA, issued exactly once
                key = (b, nb)
                if key not in xt_cache:
                    xt = xpool.tile([P, KC * NB], FP16, name=f"x_{b}_{nb}",
                                    tag="xt", bufs=o["xt_bufs"])
                    xt_cache[key] = xt
                    if not defer_dma:
                        col0 = (b * QB + nb) * KC * NB
                        nc.sync.dma_start(xt[:], xr[:, col0:col0 + KC * NB])
                return xt_cache[key]

            def ph1_gen(b, nb, warm=False):
                xt = get_xt(b, nb, defer_dma=True) if warm else get_xt(b, nb)
                col0 = (b * QB + nb) * KC * NB
                if warm:
                    # chunked loads interleaved with weight chunks so the PE
                    # can start after the first x+w chunk pair; the first few
                    # k-chunks load individually to minimize time-to-first-mm
                    if o["fine_warm"]:
                        kranges = [(0, 1), (1, 2), (2, 4), (4, 6), (6, 8),
                                   (8, 10), (10, 12), (12, 14), (14, 16)]
                    else:
                        kranges = [(4 * c, 4 * c + 4) for c in range(4)]
                    xring = nc.scalar if o["warm_xring"] in ("vector", "scalar") else nc.sync
                    for k0, k1 in kranges:
                        nc.sync.dma_start(wsb[:, k0 * 4 * DK:k1 * 4 * DK],
                                          wq[:, k0 * 4 * DK:k1 * 4 * DK])
                        xring.dma_start(xt[:, k0 * NB:k1 * NB],
                                        xr[:, col0 + k0 * NB:col0 + k1 * NB])
                    nc.sync.dma_start(wsb[:, QKW:], wq[:, QKW:])
                    get_xt(b, nb + 1)  # prefetch next block behind warm loads
                    pe_filler(o["fillers"])
                    if not o["late_wo"]:
                        nc.sync.dma_start(maskd[:], maskt[:, :])
                        nc.sync.dma_start(wosb[:], wo[:, :])
                else:
                    if o["late_wo"] and b == 0 and nb == 1:
                        nc.sync.dma_start(maskd[:], maskt[:, :])
                        nc.sync.dma_start(wosb[:], wo[:, :])
                tsl = slice(nb * NB, (nb + 1) * NB)
                QT, KT, V = QTb[b], KTb[b], Vb[b]
                mdest = [QT[0], KT[0], QT[1], KT[1]]
                if warm:
                    # chunk-major: all 6 PSUM groups live, consume x chunks
                    # as they arrive
                    psm = [pspool.tile([P, NB], FP32, name=f"p1w_{m}",
                                       tag=("ps_s" if m < 3 else "ps_y"),
                                       bufs=(o["pss_bufs"] if m < 3 else o["py_bufs"]))
                           for m in range(4)]
                    psv = [pspool.tile([P, NB], FP32, name=f"p1wv_{h}",
                                       tag="ps_o", bufs=o["po_bufs"])
                           for h in range(HPC)]
                    krf = dict(tuple(map(int, kv.split(":")))
                               for kv in o["kr_fillers"].split(",") if kv)
                    for ki, (k0, k1) in enumerate(kranges):
                        for m in range(4):
                            for k in range(k0, k1):
                                nc.tensor.matmul(psm[m][:], wslice(m, k),
                                                 xt[:, k * NB:(k + 1) * NB],
                                                 start=(k == 0), stop=(k == KC - 1))
                        pe_filler(krf.get(ki, 0))
                        if not o["warm_v_steady"]:
                            for h in range(HPC):
                                for q in range(4):
                                    qs = slice(q * P, (q + 1) * P)
                                    for k in range(k0, k1):
                                        nc.tensor.matmul(
                                            psv[h][:, qs],
                                            xt[:, k * NB + q * P:k * NB + (q + 1) * P],
                                            wslice(4 + h, k),
                                            start=(k == 0), stop=(k == KC - 1),
                                            skip_group_check=True)
                        yield
                    for m in range(4):
                        psum_copy(mdest[m][:, tsl], psm[m][:], o["ph1_copy"])
                    if o["warm_v_steady"]:
                        for h in range(HPC):
                            for q in range(4):
                                qs = slice(q * P, (q + 1) * P)
                                for k in range(KC):
                                    nc.tensor.matmul(
                                        psv[h][:, qs],
                                        xt[:, k * NB + q * P:k * NB + (q + 1) * P],
                                        wslice(4 + h, k),
                                        start=(k == 0), stop=(k == KC - 1),
                                        skip_group_check=True)
                            yield
                    for h in range(HPC):
                        psum_copy(V[h][:, tsl], psv[h][:], o["ph1_copy"])
                    yield
                else:
                    for m in range(4):
                        ps = pspool.tile([P, NB], FP32, name=f"p1_{b}_{nb}_{m}",
                                         tag="ps_s", bufs=o["pss_bufs"])
                        for k in range(KC):
                            nc.tensor.matmul(ps[:], wslice(m, k),
                                             xt[:, k * NB:(k + 1) * NB],
                                             start=(k == 0), stop=(k == KC - 1))
                        psum_copy(mdest[m][:, tsl], ps[:], o["ph1_copy"])
                        yield
                    for h in range(HPC):
                        psv = pspool.tile([P, NB], FP32, name=f"p1v_{b}_{nb}_{h}",
                                          tag="ps_o", bufs=o["po_bufs"])
                        for q in range(4):
                            qs = slice(q * P, (q + 1) * P)
                            for k in range(KC):
                                nc.tensor.matmul(
                                    psv[:, qs],
                                    xt[:, k * NB + q * P:k * NB + (q + 1) * P],
                                    wslice(4 + h, k),
                                    start=(k == 0), stop=(k == KC - 1),
                                    skip_group_check=True)
                        psum_copy(V[h][:, tsl], psv[:], o["ph1_copy"])
                        yield

            # ---------------- phase 2: attention ----------------
            def attn_gen(h, b, qb):
                QT, KT, V = QTb[b], KTb[b], Vb[b]
                # key tiles: 4*qb full-width off-diagonal, then 4 diagonal
                # tiles with shrinking query range
                tiles = [(kt, NB, 0, False) for kt in range(4 * qb)]
                tiles += [(4 * qb + j, NB - j * P, j * P, True) for j in range(4)]
                n = len(tiles)
                q0 = qb * NB
                ps_o = pspool.tile([P, NB], FP32, name=f"po_{b}_{h}_{qb}",
                                   tag="ps_o", bufs=o["po_bufs"])
                acc = apool.tile([P, NB], FP16, name=f"acc_{b}_{h}_{qb}",
                                 tag=f"acc{h}", bufs=o["acc_bufs"])
                pss = {}

                def issue_scores(i):
                    kt, w, qo, diag = tiles[i]
                    ps = pspool.tile([P, NB], FP32, name=f"pss_{b}_{h}_{qb}_{kt}",
                                     tag="ps_s", bufs=o["pss_bufs"])
                    ksl = slice(kt * P, (kt + 1) * P)
                    nc.tensor.matmul(ps[:, 0:w], KT[h][:, ksl],
                                     QT[h][:, q0 + qo:q0 + qo + w],
                                     start=True, stop=True)
                    pss[i] = ps

                for i in range(min(o["skew"], n)):
                    issue_scores(i)
                for i in range(n):
                    kt, w, qo, diag = tiles[i]
                    ps_s = pss.pop(i)
                    ex = apool.tile([P, NB], FP16, name=f"ex_{b}_{h}_{qb}_{kt}",
                                    tag="ex", bufs=o["ex_bufs"])
                    nc.scalar.activation(ex[:, 0:w], ps_s[:, 0:w],
                                         mybir.ActivationFunctionType.Exp,
                                         scale=SCALE)
                    if diag:
                        meng = nc.gpsimd if o["mask_eng"] == "gpsimd" else nc.vector
                        meng.tensor_mul(ex[:, 0:P], ex[:, 0:P], maskd[:])
                    if i + o["skew"] < n:
                        issue_scores(i + o["skew"])
                    ksl = slice(kt * P, (kt + 1) * P)
                    nc.tensor.matmul(ps_o[:, qo:NB], V[h][:, ksl], ex[:, 0:w],
                                     start=(i == 0), stop=(i == n - 1),
                                     skip_group_check=True)
                    if i == 0:
                        nc.vector.tensor_copy(acc[:, 0:w], ex[:, 0:w])
                    else:
                        nc.vector.tensor_add(acc[:, qo:NB], acc[:, qo:NB],
                                             ex[:, 0:w])
                    yield
                # softmax denominator + normalize
                osl = slice(q0, q0 + NB)
                if o["pool_denom"]:
                    # all-partition sum on the idle Pool engine; reciprocal on
                    # DVE costs by free size only, so take it on all 128 rows
                    dn = apool.tile([P, NB], FP32, name=f"dn_{b}_{h}_{qb}",
                                    tag=f"dn{h}", bufs=2)
                    nc.gpsimd.partition_all_reduce(dn[:], acc[:], channels=P,
                                                   reduce_op=bass_isa.ReduceOp.add)
                    bc = apool.tile([P, NB], FP32, name=f"bc_{b}_{h}_{qb}",
                                    tag=f"bc{h}", bufs=2)
                    with nc.allow_low_precision(reason="fp32 softmax recip"):
                        nc.vector.reciprocal(bc[:], dn[:])
                    yield
                    nc.vector.tensor_mul(outT[h][:, osl], ps_o[:], bc[:])
                    yield
                    return
                ps_d = pspool.tile([1, NB], FP32, name=f"pd_{b}_{h}_{qb}",
                                   tag="ps_y", bufs=o["py_bufs"])
                nc.tensor.matmul(ps_d[:1, :], onc[:, :], acc[:],
                                 start=True, stop=True)
                rec = apool.tile([1, NB], FP16, name=f"rec_{b}_{h}_{qb}",
                                 tag=f"rec{h}", bufs=2)
                with nc.allow_low_precision(reason="fp16 softmax recip"):
                    nc.vector.reciprocal(rec[:1, :], ps_d[:1, :])
                yield
                ps_b = pspool.tile([P, NB], FP32, name=f"pb_{b}_{h}_{qb}",
                                   tag="ps_y", bufs=o["py_bufs"])
                nc.tensor.matmul(ps_b[:], onr[:1, :], rec[:1, :],
                                 start=True, stop=True)
                bc = apool.tile([P, NB], FP32, name=f"bc_{b}_{h}_{qb}",
                                tag="bc", bufs=2)
                if o["bc_eng"] == "gpsimd":
                    nc.gpsimd.tensor_copy(bc[:], ps_b[:])
                else:
                    nc.vector.tensor_copy(bc[:], ps_b[:])
                nc.vector.tensor_mul(outT[h][:, osl], ps_o[:], bc[:])
                yield

            # ---------------- phase 3: output projection ----------------
            def proj_gen(b, qb, tail=False):
                last_tt = 4 * qb + 3
                eb_tts = {"eb": set(range(4 * qb, 4 * qb + 4)),
                          "lasttt": {last_tt},
                          "last2": {last_tt - 1, last_tt},
                          "tt": set()}[o["tail_dma"]] if tail else set()
                tags = ([("ps_y", o["py_bufs"]), ("ps_s", o["pss_bufs"]),
                         ("ps_o", o["po_bufs"])] if tail and o["tail_rotate"]
                        else [("ps_y", o["py_bufs"])])
                ti = 0
                for tt in range(4 * qb, 4 * qb + 4):
                    yt = apool.tile([P, D], FP16, name=f"yt_{b}_{tt}",
                                    tag="yt", bufs=o["yt_bufs"])
                    csl = slice(tt * P, (tt + 1) * P)
                    for eb in range(4):
                        tag, tb = tags[ti % len(tags)]
                        ti += 1
                        ps = pspool.tile([P, NB], FP32, name=f"py_{b}_{tt}_{eb}",
                                         tag=tag, bufs=tb)
                        for h in range(HPC):
                            nc.tensor.matmul(ps[:], outT[h][:, csl],
                                             wosb[:, h * D + eb * NB:h * D + (eb + 1) * NB],
                                             start=(h == 0), stop=(h == HPC - 1))
                        psum_copy(yt[:, eb * NB:(eb + 1) * NB], ps[:],
                                  "sv" if tail else o["proj_copy"])
                        if tt in eb_tts:
                            nc.sync.dma_start(
                                y[b * T + tt * P:b * T + (tt + 1) * P,
                                  eb * NB:(eb + 1) * NB],
                                yt[:, eb * NB:(eb + 1) * NB])
                        yield
                    if tt not in eb_tts:
                        nc.sync.dma_start(y[b * T + tt * P:b * T + (tt + 1) * P, :],
                                          yt[:])
                    yield

            def rr(gens, delays=None):
                delays = delays or [0] * len(gens)
                done = [False] * len(gens)
                rounds = 0
                while not all(done):
                    for gi, g in enumerate(gens):
                        if done[gi] or rounds < delays[gi]:
                            continue
                        try:
                            next(g)
                        except StopIteration:
                            done[gi] = True
                    rounds += 1

            # ---------------- driver ----------------
            pending_proj = None  # proj generator deferred across qb/batch
            if o["ilv_ph1"]:
                # software-pipeline: ph1 of block nb+1 (or next batch's nb0)
                # interleaves with attention of block qb=nb
                rr([ph1_gen(0, 0, warm=o["warm"])])
                for b in range(B):
                    for qb in range(QB):
                        gens = []
                        delays = []
                        ph1g = None
                        if qb < QB - 1:
                            ph1g = ph1_gen(b, qb + 1)
                        elif b + 1 < B:
                            ph1g = ph1_gen(b + 1, 0)
                        if ph1g is not None and not o["attn_first"]:
                            gens.append(ph1g)
                            delays.append(o["ph1_delay"])
                            ph1g = None
                        gens += [attn_gen(0, b, qb), attn_gen(1, b, qb)]
                        delays += [0, 0]
                        if ph1g is not None:
                            gens.append(ph1g)
                            delays.append(o["ph1_delay"])
                        if pending_proj is not None:
                            gens.append(pending_proj)
                            delays.append(o["proj_delay_tail"]
                                          if (b == B - 1 and qb == QB - 1)
                                          else o["proj_delay"])
                        rr(gens, delays)
                        pending_proj = proj_gen(b, qb,
                                                tail=(b == B - 1 and qb == QB - 1))
                    if debug:
                        for nm, t in (("QT0", QTb[b][0]), ("KT0", KTb[b][0]),
                                      ("V0", Vb[b][0]), ("outT0", outT[0])):
                            nc.sync.dma_start(dbg[nm][:, b * T:(b + 1) * T], t[:])
                rr([pending_proj])
            else:
                for b in range(B):
                    for nb in range(QB):
                        g = ph1_gen(b, nb, warm=(o["warm"] and b == 0 and nb == 0))
                        if pending_proj is not None and o["rr_ph1"]:
                            rr([g, pending_proj])
                            pending_proj = None
                        else:
                            rr([g])
                    if pending_proj is not None:
                        rr([pending_proj])
                        pending_proj = None
                    for qb in range(QB):
                        gens = [attn_gen(0, b, qb), attn_gen(1, b, qb)]
                        delays = [0, 0]
                        if pending_proj is not None:
                            gens.append(pending_proj)
                            delays.append(o["proj_delay"])
                        rr(gens, delays)
                        pending_proj = proj_gen(b, qb, tail=(b == B - 1 and qb == QB - 1))
                    if debug:
                        for nm, t in (("QT0", QTb[b][0]), ("KT0", KTb[b][0]),
                                      ("V0", Vb[b][0]), ("outT0", outT[0])):
                            nc.sync.dma_start(dbg[nm][:, b * T:(b + 1) * T], t[:])
                rr([pending_proj])

    nc.compile()
    return nc


def prep_inputs(x, w_qkv, w_o):
    """Host-side shard prep. Returns per-core input maps."""
    x = np.asarray(x, np.float32).reshape(B, T, D)
    # xr[p, (b, nb, k, q)] = x[b, nb*NB+q, k*P+p]
    xr = x.reshape(B, QB, NB, KC, P).transpose(4, 0, 1, 3, 2)
    xr = np.ascontiguousarray(xr.reshape(P, B * QB * KC * NB)).astype(np.float16)
    w_qkv = np.asarray(w_qkv, np.float32)
    w_o = np.asarray(w_o, np.float32)

    mask = (np.arange(P)[:, None] <= np.arange(P)[None, :]).astype(np.float16)

    in_maps = []
    for c in range(NCORES):
        h0, h1 = HPC * c, HPC * c + 1
        blocks = [w_qkv[h0 * DK:(h0 + 1) * DK],               # Q h0
                  w_qkv[D + h0 * DK:D + (h0 + 1) * DK],       # K h0
                  w_qkv[h1 * DK:(h1 + 1) * DK],               # Q h1
                  w_qkv[D + h1 * DK:D + (h1 + 1) * DK],       # K h1
                  w_qkv[2 * D + h0 * DK:2 * D + (h0 + 1) * DK],  # V h0
                  w_qkv[2 * D + h1 * DK:2 * D + (h1 + 1) * DK]]  # V h1
        wstack = np.stack(blocks)                   # [6, DK(j), D(d)]
        t = wstack.reshape(6, DK, KC, P)            # (m, j, k, p)
        qk = t[:4].transpose(3, 2, 0, 1).reshape(P, KC * 4 * DK)   # (p, k, m, j)
        vv = t[4:].transpose(3, 2, 0, 1).reshape(P, KC * 2 * DK)
        wq_host = np.ascontiguousarray(
            np.concatenate([qk, vv], axis=1)).astype(np.float16)
        wof = w_o[:, c * HPC * DK:(c + 1) * HPC * DK]  # [D(e), 256(f)]
        wo_host = np.ascontiguousarray(
            wof.T.reshape(HPC, P, D).transpose(1, 0, 2).reshape(P, HPC * D)
        ).astype(np.float16)
        in_maps.append({"xr": xr, "wq": wq_host, "wo": wo_host, "maskt": mask})
    return in_maps


_nc_cache = {}


def get_nc(debug=False, **opts):
    key = (debug, tuple(sorted(opts.items())))
    if key not in _nc_cache:
        _nc_cache[key] = build(debug=debug, **opts)
    return _nc_cache[key]


def run(x, w_qkv, w_o, debug=False, **opts):
    nc = get_nc(debug=debug, **opts)
    in_maps = prep_inputs(x, w_qkv, w_o)
    res = bass_utils.run_bass_kernel_spmd(nc, in_maps, core_ids=list(range(NCORES)))
    return res


def kernel(x, w_qkv, w_o):
    res = run(x, w_qkv, w_o)
    y = res.results[0]["y"].astype(np.float64)
    for c in range(1, NCORES):
        y += res.results[c]["y"]
    return y.astype(np.float32).reshape(B, T, D)



# revision 54
# speedup vs baseline: 1.0013x; 1.0006x over previous
"""Trainium2 Bass kernel for causal multi-head attention (dense transformer block).

Math (reference semantics):
    qkv = x @ w_qkv.T ; split into Q,K,V heads [B,H,T,dk]
    (rotary in the reference rotates Q and K of head h by a constant,
     time-independent orthogonal rotation R_h; since scores = (R_h q)·(R_h k)
     = q·k, the rotation cancels exactly and is skipped here)
    scores = causal_mask(Q @ K.T / sqrt(dk)); attn = softmax(scores)
    out = attn @ V ; y = out @ w_o.T
Sharding: head-parallel over 8 cores (2 heads/core, both batches). Each core
computes a partial y (its heads' contribution through w_o columns); the host
sums the 8 partials (the "all-reduce").

Design (everything fp16 on-chip, fp32 PSUM accumulation; host pre-packs
x/w/wo into layouts that make every load a single contiguous DMA):
  phase 1 per (b, nb): one batched x DMA; Q,K as [dk, tok] via m-major
    accumulation; V computed DIRECTLY in [tok, dk] layout (x-chunk stationary)
    so no PE transposes are needed. QT/KT/V double-buffered across batches.
  phase 2 per (b, qb): causal-exact diagonal tiles with shrinking free size
    (full PE rate at any ap since fp16); softmax denominator accumulated on
    DVE in fp16 (acc += ex per key-tile), summed across partitions on the
    otherwise-idle Pool engine (partition_all_reduce), reciprocal on DVE over
    all 128 rows (DVE cost depends on free size only) — softmax costs the PE
    nothing and takes no PSUM banks.
  schedule: generator round-robin interleaves, per query block qb, the two
    heads' attention, the deferred output projection of block qb-1, and
    phase 1 of block nb=qb+1 (or the next batch's first block) — the PE queue
    always holds ready work across exp/reciprocal latencies.
  phase 3: wide [128, 2048] y stores, one DMA per token tile (per-eb DMAs on
    the last tile to shorten the final drain).
"""

import contextlib

import numpy as np

import concourse.bacc as bacc
import concourse.bass as bass
import concourse.bass_isa as bass_isa
import concourse.mybir as mybir
import concourse.tile as tile
from concourse import bass_utils

B, T, D, H, DK = 2, 2048, 2048, 16, 128
NCORES = 8
HPC = H // NCORES  # heads per core
P = 128
NB = 512           # token/query block
KC = D // P        # 16 contraction chunks
QB = T // NB       # 4 query blocks per batch
NT = T // P        # 16 token tiles per batch
FP32 = mybir.dt.float32
FP16 = mybir.dt.float16
SCALE = 1.0 / np.sqrt(DK)

DEFAULT_OPTS = dict(
    skew=2, ex_bufs=8, pss_bufs=4, po_bufs=2, py_bufs=2, yt_bufs=4,
    acc_bufs=2, xt_bufs=2, proj_delay=2, bc_direct=False, warm_chunks=4,
    rr_ph1=True, warm=True, warm_v_steady=True, pool_copy=False,
    bc_eng="vector", denom_split=False, tail_rotate=True, fine_warm=True,
    pool_denom=True, tail_dma="lasttt", ph1_copy="sv", proj_copy="sv",
    mask_eng="vector", late_wo=True, ilv_ph1=True, warm_xring="sync",
    proj_delay_tail=7, fillers=0, kr_fillers="", attn_first=False, ph1_delay=0,
    acc_eng="vector", proj_copy_last="sv",
)


def build(debug=False, **opts):
    o = dict(DEFAULT_OPTS); o.update(opts)
    nc = bacc.Bacc("TRN2", target_bir_lowering=False, debug=False,
                   num_devices=NCORES)
    xr = nc.dram_tensor("xr", [P, B * QB * KC * NB], FP16, kind="ExternalInput")
    wq = nc.dram_tensor("wq", [P, KC * 6 * DK], FP16, kind="ExternalInput")
    wo = nc.dram_tensor("wo", [P, HPC * D], FP16, kind="ExternalInput")
    maskt = nc.dram_tensor("maskt", [P, P], FP16, kind="ExternalInput")
    y = nc.dram_tensor("y", [B * T, D], FP16, kind="ExternalOutput")
    dbg = {}
    if debug:
        for nm in ("QT0", "KT0", "V0", "outT0"):
            dbg[nm] = nc.dram_tensor("dbg_" + nm, [P, B * T], FP16,
                                     kind="ExternalOutput")

    with tile.TileContext(nc) as tc:
        with (
            tc.tile_pool(name="const", bufs=1) as cpool,
            tc.tile_pool(name="xp", bufs=2) as xpool,
            tc.tile_pool(name="qkv", bufs=1) as qpool,
            tc.tile_pool(name="attn", bufs=1) as apool,
            tc.tile_pool(name="ps", bufs=1, space="PSUM") as pspool,
        ):
            wsb = cpool.tile([P, KC * 6 * DK], FP16, name="wsb")
            wosb = cpool.tile([P, HPC * D], FP16, name="wosb")
            maskd = cpool.tile([P, P], FP16, name="maskd")
            onc = cpool.tile([P, 1], FP16, name="onc")
            onr = cpool.tile([1, P], FP16, name="onr")
            nc.vector.memset(onc[:], 1.0)
            nc.vector.memset(onr[:], 1.0)

            # per-batch per-head tensors: QT/KT/V double-buffered across
            # batches so next-batch phase 1 can interleave with the current
            # batch's last attention block without clobbering its K/V reads.
            QTb = [[qpool.tile([P, T], FP16, name=f"QT{h}_{b}") for h in range(HPC)]
                   for b in range(B)]
            KTb = [[qpool.tile([P, T], FP16, name=f"KT{h}_{b}") for h in range(HPC)]
                   for b in range(B)]
            Vb = [[qpool.tile([P, T], FP16, name=f"V{h}_{b}") for h in range(HPC)]
                  for b in range(B)]
            # outT is safe single-buffered: proj(b,qb) only reads its own
            # qb column block, disjoint from the next batch's writes
            outT = [qpool.tile([P, T], FP16, name=f"oT{h}") for h in range(HPC)]

            # lhsT slices for phase 1: (m, k) -> weight chunk [128 d, 128 dk]
            # layout: QK region first (k-major, Qh0 Kh0 Qh1 Kh1 per k), then
            # V region (k-major, Vh0 Vh1 per k) so warm-start QK loads are
            # small and V weights arrive in one deferred DMA
            QKW = KC * 4 * DK  # 8192: start of the V region

            def wslice(m, k):
                if m < 4:
                    return wsb[:, k * 4 * DK + m * DK:k * 4 * DK + (m + 1) * DK]
                c0 = QKW + k * 2 * DK + (m - 4) * DK
                return wsb[:, c0:c0 + DK]

            fill_state = {"tile": None, "dum": None, "n": 0}

            def pe_filler(count):
                # dependency-free matmuls that keep the PE p-state ramped
                # while real work is DMA-gated (output never read)
                if count <= 0:
                    return
                if fill_state["dum"] is None:
                    dum = cpool.tile([P, NB], FP16, name="fill_dum")
                    nc.vector.memset(dum[:], 0.0)
                    fill_state["dum"] = dum
                    fill_state["tile"] = pspool.tile([1, NB], FP32, name="fill_ps",
                                                     tag="ps_y", bufs=o["py_bufs"])
                for _ in range(count):
                    nc.tensor.matmul(fill_state["tile"][:1, :], onc[:, :],
                                     fill_state["dum"][:], start=True, stop=True,
                                     skip_group_check=True)
                    fill_state["n"] += 1

            copy_flip = [0]
            engsets = {
                "sv": [nc.scalar, nc.vector],
                "v": [nc.vector],
                "s": [nc.scalar],
                "vp": [nc.vector, nc.gpsimd],
                "svp": [nc.scalar, nc.vector, nc.gpsimd],
                "sp": [nc.scalar, nc.gpsimd],
                "p": [nc.gpsimd],
            }

            def psum_copy(dst, src, which="sv"):
                # rotate PSUM->SBUF drains across the engine set
                engines = engsets[which]
                eng = engines[copy_flip[0] % len(engines)]
                if eng is nc.scalar:
                    eng.copy(dst, src)
                else:
                    eng.tensor_copy(dst, src)
                copy_flip[0] += 1

            # ---------------- phase 1 ----------------
            xt_cache = {}

            def get_xt(b, nb, defer_dma=False):
                # tile + (unless deferred) its load DMA, issued exactly once
                key = (b, nb)
                if key not in xt_cache:
                    xt = xpool.tile([P, KC * NB], FP16, name=f"x_{b}_{nb}",
                                    tag="xt", bufs=o["xt_bufs"])
                    xt_cache[key] = xt
                    if not defer_dma:
                        col0 = (b * QB + nb) * KC * NB
                        nc.sync.dma_start(xt[:], xr[:, col0:col0 + KC * NB])
                return xt_cache[key]

            def ph1_gen(b, nb, warm=False):
                xt = get_xt(b, nb, defer_dma=True) if warm else get_xt(b, nb)
                col0 = (b * QB + nb) * KC * NB
                if warm:
                    # chunked loads interleaved with weight chunks so the PE
                    # can start after the first x+w chunk pair; the first few
                    # k-chunks load individually to minimize time-to-first-mm
                    if o["fine_warm"] == 2:
                        kranges = [(k, k + 1) for k in range(KC)]
                    elif o["fine_warm"]:
                        kranges = [(0, 1), (1, 2), (2, 4), (4, 6), (6, 8),
                                   (8, 10), (10, 12), (12, 14), (14, 16)]
                    else:
                        kranges = [(4 * c, 4 * c + 4) for c in range(4)]
                    xring = nc.scalar if o["warm_xring"] in ("vector", "scalar") else nc.sync
                    for k0, k1 in kranges:
                        nc.sync.dma_start(wsb[:, k0 * 4 * DK:k1 * 4 * DK],
                                          wq[:, k0 * 4 * DK:k1 * 4 * DK])
                        xring.dma_start(xt[:, k0 * NB:k1 * NB],
                                        xr[:, col0 + k0 * NB:col0 + k1 * NB])
                    nc.sync.dma_start(wsb[:, QKW:], wq[:, QKW:])
                    get_xt(b, nb + 1)  # prefetch next block behind warm loads
                    pe_filler(o["fillers"])
                    if not o["late_wo"]:
                        nc.sync.dma_start(maskd[:], maskt[:, :])
                        nc.sync.dma_start(wosb[:], wo[:, :])
                else:
                    if o["late_wo"] and b == 0 and nb == 1:
                        nc.sync.dma_start(maskd[:], maskt[:, :])
                        nc.sync.dma_start(wosb[:], wo[:, :])
                tsl = slice(nb * NB, (nb + 1) * NB)
                QT, KT, V = QTb[b], KTb[b], Vb[b]
                mdest = [QT[0], KT[0], QT[1], KT[1]]
                if warm:
                    # chunk-major: all 6 PSUM groups live, consume x chunks
                    # as they arrive
                    psm = [pspool.tile([P, NB], FP32, name=f"p1w_{m}",
                                       tag=("ps_s" if m < 3 else "ps_y"),
                                       bufs=(o["pss_bufs"] if m < 3 else o["py_bufs"]))
                           for m in range(4)]
                    psv = [pspool.tile([P, NB], FP32, name=f"p1wv_{h}",
                                       tag="ps_o", bufs=o["po_bufs"])
                           for h in range(HPC)]
                    krf = dict(tuple(map(int, kv.split(":")))
                               for kv in o["kr_fillers"].split(",") if kv)
                    for ki, (k0, k1) in enumerate(kranges):
                        for m in range(4):
                            for k in range(k0, k1):
                                nc.tensor.matmul(psm[m][:], wslice(m, k),
                                                 xt[:, k * NB:(k + 1) * NB],
                                                 start=(k == 0), stop=(k == KC - 1))
                        pe_filler(krf.get(ki, 0))
                        if not o["warm_v_steady"]:
                            for h in range(HPC):
                                for q in range(4):
                                    qs = slice(q * P, (q + 1) * P)
                                    for k in range(k0, k1):
                                        nc.tensor.matmul(
                                            psv[h][:, qs],
                                            xt[:, k * NB + q * P:k * NB + (q + 1) * P],
                                            wslice(4 + h, k),
                                            start=(k == 0), stop=(k == KC - 1),
                                            skip_group_check=True)
                        yield
                    for m in range(4):
                        psum_copy(mdest[m][:, tsl], psm[m][:], o["ph1_copy"])
                    if o["warm_v_steady"]:
                        for h in range(HPC):
                            for q in range(4):
                                qs = slice(q * P, (q + 1) * P)
                                for k in range(KC):
                                    nc.tensor.matmul(
                                        psv[h][:, qs],
                                        xt[:, k * NB + q * P:k * NB + (q + 1) * P],
                                        wslice(4 + h, k),
                                        start=(k == 0), stop=(k == KC - 1),
                                        skip_group_check=True)
                            yield
                    # split V copies so attention's first AV (kt0) only waits
                    # a small [128,128] copy; h0 on Act, h1 on DVE in parallel
                    t0 = nb * NB
                    nc.scalar.copy(V[0][:, t0:t0 + P], psv[0][:, 0:P])
                    nc.vector.tensor_copy(V[1][:, t0:t0 + P], psv[1][:, 0:P])
                    nc.scalar.copy(V[0][:, t0 + P:t0 + NB], psv[0][:, P:NB])
                    nc.vector.tensor_copy(V[1][:, t0 + P:t0 + NB], psv[1][:, P:NB])
                    yield
                else:
                    for m in range(4):
                        ps = pspool.tile([P, NB], FP32, name=f"p1_{b}_{nb}_{m}",
                                         tag="ps_s", bufs=o["pss_bufs"])
                        for k in range(KC):
                            nc.tensor.matmul(ps[:], wslice(m, k),
                                             xt[:, k * NB:(k + 1) * NB],
                                             start=(k == 0), stop=(k == KC - 1))
                        psum_copy(mdest[m][:, tsl], ps[:], o["ph1_copy"])
                        yield
                    for h in range(HPC):
                        psv = pspool.tile([P, NB], FP32, name=f"p1v_{b}_{nb}_{h}",
                                          tag="ps_o", bufs=o["po_bufs"])
                        for q in range(4):
                            qs = slice(q * P, (q + 1) * P)
                            for k in range(KC):
                                nc.tensor.matmul(
                                    psv[:, qs],
                                    xt[:, k * NB + q * P:k * NB + (q + 1) * P],
                                    wslice(4 + h, k),
                                    start=(k == 0), stop=(k == KC - 1),
                                    skip_group_check=True)
                        psum_copy(V[h][:, tsl], psv[:], o["ph1_copy"])
                        yield

            # ---------------- phase 2: attention ----------------
            def attn_gen(h, b, qb):
                QT, KT, V = QTb[b], KTb[b], Vb[b]
                # key tiles: 4*qb full-width off-diagonal, then 4 diagonal
                # tiles with shrinking query range
                tiles = [(kt, NB, 0, False) for kt in range(4 * qb)]
                tiles += [(4 * qb + j, NB - j * P, j * P, True) for j in range(4)]
                n = len(tiles)
                q0 = qb * NB
                ps_o = pspool.tile([P, NB], FP32, name=f"po_{b}_{h}_{qb}",
                                   tag="ps_o", bufs=o["po_bufs"])
                acc = apool.tile([P, NB], FP16, name=f"acc_{b}_{h}_{qb}",
                                 tag=f"acc{h}", bufs=o["acc_bufs"])
                pss = {}

                def issue_scores(i):
                    kt, w, qo, diag = tiles[i]
                    ps = pspool.tile([P, NB], FP32, name=f"pss_{b}_{h}_{qb}_{kt}",
                                     tag="ps_s", bufs=o["pss_bufs"])
                    ksl = slice(kt * P, (kt + 1) * P)
                    nc.tensor.matmul(ps[:, 0:w], KT[h][:, ksl],
                                     QT[h][:, q0 + qo:q0 + qo + w],
                                     start=True, stop=True)
                    pss[i] = ps

                for i in range(min(o["skew"], n)):
                    issue_scores(i)
                for i in range(n):
                    kt, w, qo, diag = tiles[i]
                    ps_s = pss.pop(i)
                    ex = apool.tile([P, NB], FP16, name=f"ex_{b}_{h}_{qb}_{kt}",
                                    tag="ex", bufs=o["ex_bufs"])
                    nc.scalar.activation(ex[:, 0:w], ps_s[:, 0:w],
                                         mybir.ActivationFunctionType.Exp,
                                         scale=SCALE)
                    if diag:
                        meng = nc.gpsimd if o["mask_eng"] == "gpsimd" else nc.vector
                        meng.tensor_mul(ex[:, 0:P], ex[:, 0:P], maskd[:])
                    if i + o["skew"] < n:
                        issue_scores(i + o["skew"])
                    ksl = slice(kt * P, (kt + 1) * P)
                    nc.tensor.matmul(ps_o[:, qo:NB], V[h][:, ksl], ex[:, 0:w],
                                     start=(i == 0), stop=(i == n - 1),
                                     skip_group_check=True)
                    aeng = nc.gpsimd if o["acc_eng"] == "gpsimd" else nc.vector
                    if i == 0:
                        aeng.tensor_copy(acc[:, 0:w], ex[:, 0:w])
                    else:
                        aeng.tensor_add(acc[:, qo:NB], acc[:, qo:NB],
                                        ex[:, 0:w])
                    yield
                # softmax denominator + normalize
                osl = slice(q0, q0 + NB)
                if o["pool_denom"]:
                    # all-partition sum on the idle Pool engine; reciprocal on
                    # DVE costs by free size only, so take it on all 128 rows
                    dn = apool.tile([P, NB], FP32, name=f"dn_{b}_{h}_{qb}",
                                    tag=f"dn{h}", bufs=2)
                    nc.gpsimd.partition_all_reduce(dn[:], acc[:], channels=P,
                                                   reduce_op=bass_isa.ReduceOp.add)
                    bc = apool.tile([P, NB], FP32, name=f"bc_{b}_{h}_{qb}",
                                    tag=f"bc{h}", bufs=2)
                    with nc.allow_low_precision(reason="fp32 softmax recip"):
                        nc.vector.reciprocal(bc[:], dn[:])
                    yield
                    nc.vector.tensor_mul(outT[h][:, osl], ps_o[:], bc[:])
                    yield
                    return
                ps_d = pspool.tile([1, NB], FP32, name=f"pd_{b}_{h}_{qb}",
                                   tag="ps_y", bufs=o["py_bufs"])
                nc.tensor.matmul(ps_d[:1, :], onc[:, :], acc[:],
                                 start=True, stop=True)
                rec = apool.tile([1, NB], FP16, name=f"rec_{b}_{h}_{qb}",
                                 tag=f"rec{h}", bufs=2)
                with nc.allow_low_precision(reason="fp16 softmax recip"):
                    nc.vector.reciprocal(rec[:1, :], ps_d[:1, :])
                yield
                ps_b = pspool.tile([P, NB], FP32, name=f"pb_{b}_{h}_{qb}",
                                   tag="ps_y", bufs=o["py_bufs"])
                nc.tensor.matmul(ps_b[:], onr[:1, :], rec[:1, :],
                                 start=True, stop=True)
                bc = apool.tile([P, NB], FP32, name=f"bc_{b}_{h}_{qb}",
                                tag="bc", bufs=2)
                if o["bc_eng"] == "gpsimd":
                    nc.gpsimd.tensor_copy(bc[:], ps_b[:])
                else:
                    nc.vector.tensor_copy(bc[:], ps_b[:])
                nc.vector.tensor_mul(outT[h][:, osl], ps_o[:], bc[:])
                yield

            # ---------------- phase 3: output projection ----------------
            def proj_gen(b, qb, tail=False):
                last_tt = 4 * qb + 3
                eb_tts = {"eb": set(range(4 * qb, 4 * qb + 4)),
                          "lasttt": {last_tt},
                          "last2": {last_tt - 1, last_tt},
                          "tt": set()}[o["tail_dma"]] if tail else set()
                tags = ([("ps_y", o["py_bufs"]), ("ps_s", o["pss_bufs"]),
                         ("ps_o", o["po_bufs"])] if tail and o["tail_rotate"]
                        else [("ps_y", o["py_bufs"])])
                ti = 0
                for tt in range(4 * qb, 4 * qb + 4):
                    yt = apool.tile([P, D], FP16, name=f"yt_{b}_{tt}",
                                    tag="yt", bufs=o["yt_bufs"])
                    csl = slice(tt * P, (tt + 1) * P)
                    for eb in range(4):
                        tag, tb = tags[ti % len(tags)]
                        ti += 1
                        ps = pspool.tile([P, NB], FP32, name=f"py_{b}_{tt}_{eb}",
                                         tag=tag, bufs=tb)
                        for h in range(HPC):
                            nc.tensor.matmul(ps[:], outT[h][:, csl],
                                             wosb[:, h * D + eb * NB:h * D + (eb + 1) * NB],
                                             start=(h == 0), stop=(h == HPC - 1))
                        psum_copy(yt[:, eb * NB:(eb + 1) * NB], ps[:],
                                  "sv" if tail else
                                  (o["proj_copy_last"] if qb == QB - 1
                                   else o["proj_copy"]))
                        if tt in eb_tts:
                            nc.sync.dma_start(
                                y[b * T + tt * P:b * T + (tt + 1) * P,
                                  eb * NB:(eb + 1) * NB],
                                yt[:, eb * NB:(eb + 1) * NB])
                        yield
                    if tt not in eb_tts:
                        nc.sync.dma_start(y[b * T + tt * P:b * T + (tt + 1) * P, :],
                                          yt[:])
                    yield

            def rr(gens, delays=None):
                delays = delays or [0] * len(gens)
                done = [False] * len(gens)
                rounds = 0
                while not all(done):
                    for gi, g in enumerate(gens):
                        if done[gi] or rounds < delays[gi]:
                            continue
                        try:
                            next(g)
                        except StopIteration:
                            done[gi] = True
                    rounds += 1

            # ---------------- driver ----------------
            pending_proj = None  # proj generator deferred across qb/batch
            if o["ilv_ph1"]:
                # software-pipeline: ph1 of block nb+1 (or next batch's nb0)
                # interleaves with attention of block qb=nb
                rr([ph1_gen(0, 0, warm=o["warm"])])
                for b in range(B):
                    for qb in range(QB):
                        gens = []
                        delays = []
                        ph1g = None
                        if qb < QB - 1:
                            ph1g = ph1_gen(b, qb + 1)
                        elif b + 1 < B:
                            ph1g = ph1_gen(b + 1, 0)
                        if ph1g is not None and not o["attn_first"]:
                            gens.append(ph1g)
                            delays.append(o["ph1_delay"])
                            ph1g = None
                        gens += [attn_gen(0, b, qb), attn_gen(1, b, qb)]
                        delays += [0, 0]
                        if ph1g is not None:
                            gens.append(ph1g)
                            delays.append(o["ph1_delay"])
                        if pending_proj is not None:
                            gens.append(pending_proj)
                            delays.append(o["proj_delay_tail"]
                                          if (b == B - 1 and qb == QB - 1)
                                          else o["proj_delay"])
                        rr(gens, delays)
                        pending_proj = proj_gen(b, qb,
                                                tail=(b == B - 1 and qb == QB - 1))
                    if debug:
                        for nm, t in (("QT0", QTb[b][0]), ("KT0", KTb[b][0]),
                                      ("V0", Vb[b][0]), ("outT0", outT[0])):
                            nc.sync.dma_start(dbg[nm][:, b * T:(b + 1) * T], t[:])
                rr([pending_proj])
            else:
                for b in range(B):
                    for nb in range(QB):
                        g = ph1_gen(b, nb, warm=(o["warm"] and b == 0 and nb == 0))
                        if pending_proj is not None and o["rr_ph1"]:
                            rr([g, pending_proj])
                            pending_proj = None
                        else:
                            rr([g])
                    if pending_proj is not None:
                        rr([pending_proj])
                        pending_proj = None
                    for qb in range(QB):
                        gens = [attn_gen(0, b, qb), attn_gen(1, b, qb)]
                        delays = [0, 0]
                        if pending_proj is not None:
                            gens.append(pending_proj)
                            delays.append(o["proj_delay"])
                        rr(gens, delays)
                        pending_proj = proj_gen(b, qb, tail=(b == B - 1 and qb == QB - 1))
                    if debug:
                        for nm, t in (("QT0", QTb[b][0]), ("KT0", KTb[b][0]),
                                      ("V0", Vb[b][0]), ("outT0", outT[0])):
                            nc.sync.dma_start(dbg[nm][:, b * T:(b + 1) * T], t[:])
                rr([pending_proj])

    nc.compile()
    return nc


def prep_inputs(x, w_qkv, w_o):
    """Host-side shard prep. Returns per-core input maps."""
    x = np.asarray(x, np.float32).reshape(B, T, D)
    # xr[p, (b, nb, k, q)] = x[b, nb*NB+q, k*P+p]
    xr = x.reshape(B, QB, NB, KC, P).transpose(4, 0, 1, 3, 2)
    xr = np.ascontiguousarray(xr.reshape(P, B * QB * KC * NB)).astype(np.float16)
    w_qkv = np.asarray(w_qkv, np.float32)
    w_o = np.asarray(w_o, np.float32)

    mask = (np.arange(P)[:, None] <= np.arange(P)[None, :]).astype(np.float16)

    in_maps = []
    for c in range(NCORES):
        h0, h1 = HPC * c, HPC * c + 1
        blocks = [w_qkv[h0 * DK:(h0 + 1) * DK],               # Q h0
                  w_qkv[D + h0 * DK:D + (h0 + 1) * DK],       # K h0
                  w_qkv[h1 * DK:(h1 + 1) * DK],               # Q h1
                  w_qkv[D + h1 * DK:D + (h1 + 1) * DK],       # K h1
                  w_qkv[2 * D + h0 * DK:2 * D + (h0 + 1) * DK],  # V h0
                  w_qkv[2 * D + h1 * DK:2 * D + (h1 + 1) * DK]]  # V h1
        wstack = np.stack(blocks)                   # [6, DK(j), D(d)]
        t = wstack.reshape(6, DK, KC, P)            # (m, j, k, p)
        qk = t[:4].transpose(3, 2, 0, 1).reshape(P, KC * 4 * DK)   # (p, k, m, j)
        vv = t[4:].transpose(3, 2, 0, 1).reshape(P, KC * 2 * DK)
        wq_host = np.ascontiguousarray(
            np.concatenate([qk, vv], axis=1)).astype(np.float16)
        wof = w_o[:, c * HPC * DK:(c + 1) * HPC * DK]  # [D(e), 256(f)]
        wo_host = np.ascontiguousarray(
            wof.T.reshape(HPC, P, D).transpose(1, 0, 2).reshape(P, HPC * D)
        ).astype(np.float16)
        in_maps.append({"xr": xr, "wq": wq_host, "wo": wo_host, "maskt": mask})
    return in_maps


_nc_cache = {}


def get_nc(debug=False, **opts):
    key = (debug, tuple(sorted(opts.items())))
    if key not in _nc_cache:
        _nc_cache[key] = build(debug=debug, **opts)
    return _nc_cache[key]


def run(x, w_qkv, w_o, debug=False, **opts):
    nc = get_nc(debug=debug, **opts)
    in_maps = prep_inputs(x, w_qkv, w_o)
    res = bass_utils.run_bass_kernel_spmd(nc, in_maps, core_ids=list(range(NCORES)))
    return res


def kernel(x, w_qkv, w_o):
    res = run(x, w_qkv, w_o)
    y = res.results[0]["y"].astype(np.float64)
    for c in range(1, NCORES):
        y += res.results[c]["y"]
    return y.astype(np.float32).reshape(B, T, D)



# revision 56
# speedup vs baseline: 1.0014x; 1.0002x over previous
"""Trainium2 Bass kernel for causal multi-head attention (dense transformer block).

Math (reference semantics):
    qkv = x @ w_qkv.T ; split into Q,K,V heads [B,H,T,dk]
    (rotary in the reference rotates Q and K of head h by a constant,
     time-independent orthogonal rotation R_h; since scores = (R_h q)·(R_h k)
     = q·k, the rotation cancels exactly and is skipped here)
    scores = causal_mask(Q @ K.T / sqrt(dk)); attn = softmax(scores)
    out = attn @ V ; y = out @ w_o.T
Sharding: head-parallel over 8 cores (2 heads/core, both batches). Each core
computes a partial y (its heads' contribution through w_o columns); the host
sums the 8 partials (the "all-reduce").

Design (everything fp16 on-chip, fp32 PSUM accumulation; host pre-packs
x/w/wo into layouts that make every load a single contiguous DMA):
  phase 1 per (b, nb): one batched x DMA; Q,K as [dk, tok] via m-major
    accumulation; V computed DIRECTLY in [tok, dk] layout (x-chunk stationary)
    so no PE transposes are needed. QT/KT/V double-buffered across batches.
  phase 2 per (b, qb): causal-exact diagonal tiles with shrinking free size
    (full PE rate at any ap since fp16); softmax denominator accumulated on
    DVE in fp16 (acc += ex per key-tile), summed across partitions on the
    otherwise-idle Pool engine (partition_all_reduce), reciprocal on DVE over
    all 128 rows (DVE cost depends on free size only) — softmax costs the PE
    nothing and takes no PSUM banks.
  schedule: generator round-robin interleaves, per query block qb, the two
    heads' attention, the deferred output projection of block qb-1, and
    phase 1 of block nb=qb+1 (or the next batch's first block) — the PE queue
    always holds ready work across exp/reciprocal latencies.
  phase 3: wide [128, 2048] y stores, one DMA per token tile (per-eb DMAs on
    the last tile to shorten the final drain).
"""

import contextlib

import numpy as np

import concourse.bacc as bacc
import concourse.bass as bass
import concourse.bass_isa as bass_isa
import concourse.mybir as mybir
import concourse.tile as tile
from concourse import bass_utils

B, T, D, H, DK = 2, 2048, 2048, 16, 128
NCORES = 8
HPC = H // NCORES  # heads per core
P = 128
NB = 512           # token/query block
KC = D // P        # 16 contraction chunks
QB = T // NB       # 4 query blocks per batch
NT = T // P        # 16 token tiles per batch
FP32 = mybir.dt.float32
FP16 = mybir.dt.float16
SCALE = 1.0 / np.sqrt(DK)

DEFAULT_OPTS = dict(
    skew=2, ex_bufs=8, pss_bufs=4, po_bufs=2, py_bufs=2, yt_bufs=4,
    acc_bufs=2, xt_bufs=2, proj_delay=2, bc_direct=False, warm_chunks=4,
    rr_ph1=True, warm=True, warm_v_steady=True, pool_copy=False,
    bc_eng="vector", denom_split=False, tail_rotate=True, fine_warm=True,
    pool_denom=True, tail_dma="lasttt", ph1_copy="sv", proj_copy="sv",
    mask_eng="vector", late_wo=True, ilv_ph1=True, warm_xring="scalar",
    proj_delay_tail=7, fillers=0, kr_fillers="", attn_first=False, ph1_delay=0,
    acc_eng="vector", proj_copy_last="sv",
    warm_kb="0,1,2,4,6,8,10,12,14,16",
)


def build(debug=False, **opts):
    o = dict(DEFAULT_OPTS); o.update(opts)
    nc = bacc.Bacc("TRN2", target_bir_lowering=False, debug=False,
                   num_devices=NCORES)
    xr = nc.dram_tensor("xr", [P, B * QB * KC * NB], FP16, kind="ExternalInput")
    wq = nc.dram_tensor("wq", [P, KC * 6 * DK], FP16, kind="ExternalInput")
    wo = nc.dram_tensor("wo", [P, HPC * D], FP16, kind="ExternalInput")
    maskt = nc.dram_tensor("maskt", [P, P], FP16, kind="ExternalInput")
    y = nc.dram_tensor("y", [B * T, D], FP16, kind="ExternalOutput")
    dbg = {}
    if debug:
        for nm in ("QT0", "KT0", "V0", "outT0"):
            dbg[nm] = nc.dram_tensor("dbg_" + nm, [P, B * T], FP16,
                                     kind="ExternalOutput")

    with tile.TileContext(nc) as tc:
        with (
            tc.tile_pool(name="const", bufs=1) as cpool,
            tc.tile_pool(name="xp", bufs=2) as xpool,
            tc.tile_pool(name="qkv", bufs=1) as qpool,
            tc.tile_pool(name="attn", bufs=1) as apool,
            tc.tile_pool(name="ps", bufs=1, space="PSUM") as pspool,
        ):
            wsb = cpool.tile([P, KC * 6 * DK], FP16, name="wsb")
            wosb = cpool.tile([P, HPC * D], FP16, name="wosb")
            maskd = cpool.tile([P, P], FP16, name="maskd")
            onc = cpool.tile([P, 1], FP16, name="onc")
            onr = cpool.tile([1, P], FP16, name="onr")
            nc.vector.memset(onc[:], 1.0)
            nc.vector.memset(onr[:], 1.0)

            # per-batch per-head tensors: QT/KT/V double-buffered across
            # batches so next-batch phase 1 can interleave with the current
            # batch's last attention block without clobbering its K/V reads.
            QTb = [[qpool.tile([P, T], FP16, name=f"QT{h}_{b}") for h in range(HPC)]
                   for b in range(B)]
            KTb = [[qpool.tile([P, T], FP16, name=f"KT{h}_{b}") for h in range(HPC)]
                   for b in range(B)]
            Vb = [[qpool.tile([P, T], FP16, name=f"V{h}_{b}") for h in range(HPC)]
                  for b in range(B)]
            # outT is safe single-buffered: proj(b,qb) only reads its own
            # qb column block, disjoint from the next batch's writes
            outT = [qpool.tile([P, T], FP16, name=f"oT{h}") for h in range(HPC)]

            # lhsT slices for phase 1: (m, k) -> weight chunk [128 d, 128 dk]
            # layout: QK region first (k-major, Qh0 Kh0 Qh1 Kh1 per k), then
            # V region (k-major, Vh0 Vh1 per k) so warm-start QK loads are
            # small and V weights arrive in one deferred DMA
            QKW = KC * 4 * DK  # 8192: start of the V region

            def wslice(m, k):
                if m < 4:
                    return wsb[:, k * 4 * DK + m * DK:k * 4 * DK + (m + 1) * DK]
                c0 = QKW + k * 2 * DK + (m - 4) * DK
                return wsb[:, c0:c0 + DK]

            fill_state = {"tile": None, "dum": None, "n": 0}

            def pe_filler(count):
                # dependency-free matmuls that keep the PE p-state ramped
                # while real work is DMA-gated (output never read)
                if count <= 0:
                    return
                if fill_state["dum"] is None:
                    dum = cpool.tile([P, NB], FP16, name="fill_dum")
                    nc.vector.memset(dum[:], 0.0)
                    fill_state["dum"] = dum
                    fill_state["tile"] = pspool.tile([1, NB], FP32, name="fill_ps",
                                                     tag="ps_y", bufs=o["py_bufs"])
                for _ in range(count):
                    nc.tensor.matmul(fill_state["tile"][:1, :], onc[:, :],
                                     fill_state["dum"][:], start=True, stop=True,
                                     skip_group_check=True)
                    fill_state["n"] += 1

            copy_flip = [0]
            engsets = {
                "sv": [nc.scalar, nc.vector],
                "v": [nc.vector],
                "s": [nc.scalar],
                "vp": [nc.vector, nc.gpsimd],
                "svp": [nc.scalar, nc.vector, nc.gpsimd],
                "sp": [nc.scalar, nc.gpsimd],
                "p": [nc.gpsimd],
            }

            def psum_copy(dst, src, which="sv"):
                # rotate PSUM->SBUF drains across the engine set
                engines = engsets[which]
                eng = engines[copy_flip[0] % len(engines)]
                if eng is nc.scalar:
                    eng.copy(dst, src)
                else:
                    eng.tensor_copy(dst, src)
                copy_flip[0] += 1

            # ---------------- phase 1 ----------------
            xt_cache = {}

            def get_xt(b, nb, defer_dma=False):
                # tile + (unless deferred) its load DMA, issued exactly once
                key = (b, nb)
                if key not in xt_cache:
                    xt = xpool.tile([P, KC * NB], FP16, name=f"x_{b}_{nb}",
                                    tag="xt", bufs=o["xt_bufs"])
                    xt_cache[key] = xt
                    if not defer_dma:
                        col0 = (b * QB + nb) * KC * NB
                        nc.sync.dma_start(xt[:], xr[:, col0:col0 + KC * NB])
                return xt_cache[key]

            def ph1_gen(b, nb, warm=False):
                xt = get_xt(b, nb, defer_dma=True) if warm else get_xt(b, nb)
                col0 = (b * QB + nb) * KC * NB
                if warm:
                    # chunked loads interleaved with weight chunks so the PE
                    # can start after the first x+w chunk pair; the first few
                    # k-chunks load individually to minimize time-to-first-mm
                    bnds = [int(v) for v in str(o["warm_kb"]).split(",")]
                    kranges = list(zip(bnds[:-1], bnds[1:]))
                    xring = nc.scalar if o["warm_xring"] in ("vector", "scalar") else nc.sync
                    for k0, k1 in kranges:
                        nc.sync.dma_start(wsb[:, k0 * 4 * DK:k1 * 4 * DK],
                                          wq[:, k0 * 4 * DK:k1 * 4 * DK])
                        xring.dma_start(xt[:, k0 * NB:k1 * NB],
                                        xr[:, col0 + k0 * NB:col0 + k1 * NB])
                    nc.sync.dma_start(wsb[:, QKW:], wq[:, QKW:])
                    get_xt(b, nb + 1)  # prefetch next block behind warm loads
                    pe_filler(o["fillers"])
                    if not o["late_wo"]:
                        nc.sync.dma_start(maskd[:], maskt[:, :])
                        nc.sync.dma_start(wosb[:], wo[:, :])
                else:
                    if o["late_wo"] and b == 0 and nb == 1:
                        nc.sync.dma_start(maskd[:], maskt[:, :])
                        nc.sync.dma_start(wosb[:], wo[:, :])
                tsl = slice(nb * NB, (nb + 1) * NB)
                QT, KT, V = QTb[b], KTb[b], Vb[b]
                mdest = [QT[0], KT[0], QT[1], KT[1]]
                if warm:
                    # chunk-major: all 6 PSUM groups live, consume x chunks
                    # as they arrive
                    psm = [pspool.tile([P, NB], FP32, name=f"p1w_{m}",
                                       tag=("ps_s" if m < 3 else "ps_y"),
                                       bufs=(o["pss_bufs"] if m < 3 else o["py_bufs"]))
                           for m in range(4)]
                    psv = [pspool.tile([P, NB], FP32, name=f"p1wv_{h}",
                                       tag="ps_o", bufs=o["po_bufs"])
                           for h in range(HPC)]
                    krf = dict(tuple(map(int, kv.split(":")))
                               for kv in o["kr_fillers"].split(",") if kv)
                    for ki, (k0, k1) in enumerate(kranges):
                        for m in range(4):
                            for k in range(k0, k1):
                                nc.tensor.matmul(psm[m][:], wslice(m, k),
                                                 xt[:, k * NB:(k + 1) * NB],
                                                 start=(k == 0), stop=(k == KC - 1))
                        pe_filler(krf.get(ki, 0))
                        if not o["warm_v_steady"]:
                            for h in range(HPC):
                                for q in range(4):
                                    qs = slice(q * P, (q + 1) * P)
                                    for k in range(k0, k1):
                                        nc.tensor.matmul(
                                            psv[h][:, qs],
                                            xt[:, k * NB + q * P:k * NB + (q + 1) * P],
                                            wslice(4 + h, k),
                                            start=(k == 0), stop=(k == KC - 1),
                                            skip_group_check=True)
                        yield
                    for m in range(4):
                        psum_copy(mdest[m][:, tsl], psm[m][:], o["ph1_copy"])
                    if o["warm_v_steady"]:
                        for h in range(HPC):
                            for q in range(4):
                                qs = slice(q * P, (q + 1) * P)
                                for k in range(KC):
                                    nc.tensor.matmul(
                                        psv[h][:, qs],
                                        xt[:, k * NB + q * P:k * NB + (q + 1) * P],
                                        wslice(4 + h, k),
                                        start=(k == 0), stop=(k == KC - 1),
                                        skip_group_check=True)
                            yield
                    # split V copies so attention's first AV (kt0) only waits
                    # a small [128,128] copy; h0 on Act, h1 on DVE in parallel
                    t0 = nb * NB
                    nc.scalar.copy(V[0][:, t0:t0 + P], psv[0][:, 0:P])
                    nc.vector.tensor_copy(V[1][:, t0:t0 + P], psv[1][:, 0:P])
                    nc.scalar.copy(V[0][:, t0 + P:t0 + NB], psv[0][:, P:NB])
                    nc.vector.tensor_copy(V[1][:, t0 + P:t0 + NB], psv[1][:, P:NB])
                    yield
                else:
                    for m in range(4):
                        ps = pspool.tile([P, NB], FP32, name=f"p1_{b}_{nb}_{m}",
                                         tag="ps_s", bufs=o["pss_bufs"])
                        for k in range(KC):
                            nc.tensor.matmul(ps[:], wslice(m, k),
                                             xt[:, k * NB:(k + 1) * NB],
                                             start=(k == 0), stop=(k == KC - 1))
                        psum_copy(mdest[m][:, tsl], ps[:], o["ph1_copy"])
                        yield
                    for h in range(HPC):
                        psv = pspool.tile([P, NB], FP32, name=f"p1v_{b}_{nb}_{h}",
                                          tag="ps_o", bufs=o["po_bufs"])
                        for q in range(4):
                            qs = slice(q * P, (q + 1) * P)
                            for k in range(KC):
                                nc.tensor.matmul(
                                    psv[:, qs],
                                    xt[:, k * NB + q * P:k * NB + (q + 1) * P],
                                    wslice(4 + h, k),
                                    start=(k == 0), stop=(k == KC - 1),
                                    skip_group_check=True)
                        psum_copy(V[h][:, tsl], psv[:], o["ph1_copy"])
                        yield

            # ---------------- phase 2: attention ----------------
            def attn_gen(h, b, qb):
                QT, KT, V = QTb[b], KTb[b], Vb[b]
                # key tiles: 4*qb full-width off-diagonal, then 4 diagonal
                # tiles with shrinking query range
                tiles = [(kt, NB, 0, False) for kt in range(4 * qb)]
                tiles += [(4 * qb + j, NB - j * P, j * P, True) for j in range(4)]
                n = len(tiles)
                q0 = qb * NB
                ps_o = pspool.tile([P, NB], FP32, name=f"po_{b}_{h}_{qb}",
                                   tag="ps_o", bufs=o["po_bufs"])
                acc = apool.tile([P, NB], FP16, name=f"acc_{b}_{h}_{qb}",
                                 tag=f"acc{h}", bufs=o["acc_bufs"])
                pss = {}

                def issue_scores(i):
                    kt, w, qo, diag = tiles[i]
                    ps = pspool.tile([P, NB], FP32, name=f"pss_{b}_{h}_{qb}_{kt}",
                                     tag="ps_s", bufs=o["pss_bufs"])
                    ksl = slice(kt * P, (kt + 1) * P)
                    nc.tensor.matmul(ps[:, 0:w], KT[h][:, ksl],
                                     QT[h][:, q0 + qo:q0 + qo + w],
                                     start=True, stop=True)
                    pss[i] = ps

                for i in range(min(o["skew"], n)):
                    issue_scores(i)
                for i in range(n):
                    kt, w, qo, diag = tiles[i]
                    ps_s = pss.pop(i)
                    ex = apool.tile([P, NB], FP16, name=f"ex_{b}_{h}_{qb}_{kt}",
                                    tag="ex", bufs=o["ex_bufs"])
                    nc.scalar.activation(ex[:, 0:w], ps_s[:, 0:w],
                                         mybir.ActivationFunctionType.Exp,
                                         scale=SCALE)
                    if diag:
                        meng = nc.gpsimd if o["mask_eng"] == "gpsimd" else nc.vector
                        meng.tensor_mul(ex[:, 0:P], ex[:, 0:P], maskd[:])
                    if i + o["skew"] < n:
                        issue_scores(i + o["skew"])
                    ksl = slice(kt * P, (kt + 1) * P)
                    nc.tensor.matmul(ps_o[:, qo:NB], V[h][:, ksl], ex[:, 0:w],
                                     start=(i == 0), stop=(i == n - 1),
                                     skip_group_check=True)
                    aeng = nc.gpsimd if o["acc_eng"] == "gpsimd" else nc.vector
                    if i == 0:
                        aeng.tensor_copy(acc[:, 0:w], ex[:, 0:w])
                    else:
                        aeng.tensor_add(acc[:, qo:NB], acc[:, qo:NB],
                                        ex[:, 0:w])
                    yield
                # softmax denominator + normalize
                osl = slice(q0, q0 + NB)
                if o["pool_denom"]:
                    # all-partition sum on the idle Pool engine; reciprocal on
                    # DVE costs by free size only, so take it on all 128 rows
                    dn = apool.tile([P, NB], FP32, name=f"dn_{b}_{h}_{qb}",
                                    tag=f"dn{h}", bufs=2)
                    nc.gpsimd.partition_all_reduce(dn[:], acc[:], channels=P,
                                                   reduce_op=bass_isa.ReduceOp.add)
                    bc = apool.tile([P, NB], FP32, name=f"bc_{b}_{h}_{qb}",
                                    tag=f"bc{h}", bufs=2)
                    with nc.allow_low_precision(reason="fp32 softmax recip"):
                        nc.vector.reciprocal(bc[:], dn[:])
                    yield
                    nc.vector.tensor_mul(outT[h][:, osl], ps_o[:], bc[:])
                    yield
                    return
                ps_d = pspool.tile([1, NB], FP32, name=f"pd_{b}_{h}_{qb}",
                                   tag="ps_y", bufs=o["py_bufs"])
                nc.tensor.matmul(ps_d[:1, :], onc[:, :], acc[:],
                                 start=True, stop=True)
                rec = apool.tile([1, NB], FP16, name=f"rec_{b}_{h}_{qb}",
                                 tag=f"rec{h}", bufs=2)
                with nc.allow_low_precision(reason="fp16 softmax recip"):
                    nc.vector.reciprocal(rec[:1, :], ps_d[:1, :])
                yield
                ps_b = pspool.tile([P, NB], FP32, name=f"pb_{b}_{h}_{qb}",
                                   tag="ps_y", bufs=o["py_bufs"])
                nc.tensor.matmul(ps_b[:], onr[:1, :], rec[:1, :],
                                 start=True, stop=True)
                bc = apool.tile([P, NB], FP32, name=f"bc_{b}_{h}_{qb}",
                                tag="bc", bufs=2)
                if o["bc_eng"] == "gpsimd":
                    nc.gpsimd.tensor_copy(bc[:], ps_b[:])
                else:
                    nc.vector.tensor_copy(bc[:], ps_b[:])
                nc.vector.tensor_mul(outT[h][:, osl], ps_o[:], bc[:])
                yield

            # ---------------- phase 3: output projection ----------------
            def proj_gen(b, qb, tail=False):
                last_tt = 4 * qb + 3
                eb_tts = {"eb": set(range(4 * qb, 4 * qb + 4)),
                          "lasttt": {last_tt},
                          "last2": {last_tt - 1, last_tt},
                          "tt": set()}[o["tail_dma"]] if tail else set()
                tags = ([("ps_y", o["py_bufs"]), ("ps_s", o["pss_bufs"]),
                         ("ps_o", o["po_bufs"])] if tail and o["tail_rotate"]
                        else [("ps_y", o["py_bufs"])])
                ti = 0
                for tt in range(4 * qb, 4 * qb + 4):
                    yt = apool.tile([P, D], FP16, name=f"yt_{b}_{tt}",
                                    tag="yt", bufs=o["yt_bufs"])
                    csl = slice(tt * P, (tt + 1) * P)
                    for eb in range(4):
                        tag, tb = tags[ti % len(tags)]
                        ti += 1
                        ps = pspool.tile([P, NB], FP32, name=f"py_{b}_{tt}_{eb}",
                                         tag=tag, bufs=tb)
                        for h in range(HPC):
                            nc.tensor.matmul(ps[:], outT[h][:, csl],
                                             wosb[:, h * D + eb * NB:h * D + (eb + 1) * NB],
                                             start=(h == 0), stop=(h == HPC - 1))
                        psum_copy(yt[:, eb * NB:(eb + 1) * NB], ps[:],
                                  "sv" if tail else
                                  (o["proj_copy_last"] if qb == QB - 1
                                   else o["proj_copy"]))
                        if tt in eb_tts:
                            nc.sync.dma_start(
                                y[b * T + tt * P:b * T + (tt + 1) * P,
                                  eb * NB:(eb + 1) * NB],
                                yt[:, eb * NB:(eb + 1) * NB])
                        yield
                    if tt not in eb_tts:
                        nc.sync.dma_start(y[b * T + tt * P:b * T + (tt + 1) * P, :],
                                          yt[:])
                    yield

            def rr(gens, delays=None):
                delays = delays or [0] * len(gens)
                done = [False] * len(gens)
                rounds = 0
                while not all(done):
                    for gi, g in enumerate(gens):
                        if done[gi] or rounds < delays[gi]:
                            continue
                        try:
                            next(g)
                        except StopIteration:
                            done[gi] = True
                    rounds += 1

            # ---------------- driver ----------------
            pending_proj = None  # proj generator deferred across qb/batch
            if o["ilv_ph1"]:
                # software-pipeline: ph1 of block nb+1 (or next batch's nb0)
                # interleaves with attention of block qb=nb
                rr([ph1_gen(0, 0, warm=o["warm"])])
                for b in range(B):
                    for qb in range(QB):
                        gens = []
                        delays = []
                        ph1g = None
                        if qb < QB - 1:
                            ph1g = ph1_gen(b, qb + 1)
                        elif b + 1 < B:
                            ph1g = ph1_gen(b + 1, 0)
                        if ph1g is not None and not o["attn_first"]:
                            gens.append(ph1g)
                            delays.append(o["ph1_delay"])
                            ph1g = None
                        gens += [attn_gen(0, b, qb), attn_gen(1, b, qb)]
                        delays += [0, 0]
                        if ph1g is not None:
                            gens.append(ph1g)
                            delays.append(o["ph1_delay"])
                        if pending_proj is not None:
                            gens.append(pending_proj)
                            delays.append(o["proj_delay_tail"]
                                          if (b == B - 1 and qb == QB - 1)
                                          else o["proj_delay"])
                        rr(gens, delays)
                        pending_proj = proj_gen(b, qb,
                                                tail=(b == B - 1 and qb == QB - 1))
                    if debug:
                        for nm, t in (("QT0", QTb[b][0]), ("KT0", KTb[b][0]),
                                      ("V0", Vb[b][0]), ("outT0", outT[0])):
                            nc.sync.dma_start(dbg[nm][:, b * T:(b + 1) * T], t[:])
                rr([pending_proj])
            else:
                for b in range(B):
                    for nb in range(QB):
                        g = ph1_gen(b, nb, warm=(o["warm"] and b == 0 and nb == 0))
                        if pending_proj is not None and o["rr_ph1"]:
                            rr([g, pending_proj])
                            pending_proj = None
                        else:
                            rr([g])
                    if pending_proj is not None:
                        rr([pending_proj])
                        pending_proj = None
                    for qb in range(QB):
                        gens = [attn_gen(0, b, qb), attn_gen(1, b, qb)]
                        delays = [0, 0]
                        if pending_proj is not None:
                            gens.append(pending_proj)
                            delays.append(o["proj_delay"])
                        rr(gens, delays)
                        pending_proj = proj_gen(b, qb, tail=(b == B - 1 and qb == QB - 1))
                    if debug:
                        for nm, t in (("QT0", QTb[b][0]), ("KT0", KTb[b][0]),
                                      ("V0", Vb[b][0]), ("outT0", outT[0])):
                            nc.sync.dma_start(dbg[nm][:, b * T:(b + 1) * T], t[:])
                rr([pending_proj])

    nc.compile()
    return nc


def prep_inputs(x, w_qkv, w_o):
    """Host-side shard prep. Returns per-core input maps."""
    x = np.asarray(x, np.float32).reshape(B, T, D)
    # xr[p, (b, nb, k, q)] = x[b, nb*NB+q, k*P+p]
    xr = x.reshape(B, QB, NB, KC, P).transpose(4, 0, 1, 3, 2)
    xr = np.ascontiguousarray(xr.reshape(P, B * QB * KC * NB)).astype(np.float16)
    w_qkv = np.asarray(w_qkv, np.float32)
    w_o = np.asarray(w_o, np.float32)

    mask = (np.arange(P)[:, None] <= np.arange(P)[None, :]).astype(np.float16)

    in_maps = []
    for c in range(NCORES):
        h0, h1 = HPC * c, HPC * c + 1
        blocks = [w_qkv[h0 * DK:(h0 + 1) * DK],               # Q h0
                  w_qkv[D + h0 * DK:D + (h0 + 1) * DK],       # K h0
                  w_qkv[h1 * DK:(h1 + 1) * DK],               # Q h1
                  w_qkv[D + h1 * DK:D + (h1 + 1) * DK],       # K h1
                  w_qkv[2 * D + h0 * DK:2 * D + (h0 + 1) * DK],  # V h0
                  w_qkv[2 * D + h1 * DK:2 * D + (h1 + 1) * DK]]  # V h1
        wstack = np.stack(blocks)                   # [6, DK(j), D(d)]
        t = wstack.reshape(6, DK, KC, P)            # (m, j, k, p)
        qk = t[:4].transpose(3, 2, 0, 1).reshape(P, KC * 4 * DK)   # (p, k, m, j)
        vv = t[4:].transpose(3, 2, 0, 1).reshape(P, KC * 2 * DK)
        wq_host = np.ascontiguousarray(
            np.concatenate([qk, vv], axis=1)).astype(np.float16)
        wof = w_o[:, c * HPC * DK:(c + 1) * HPC * DK]  # [D(e), 256(f)]
        wo_host = np.ascontiguousarray(
            wof.T.reshape(HPC, P, D).transpose(1, 0, 2).reshape(P, HPC * D)
        ).astype(np.float16)
        in_maps.append({"xr": xr, "wq": wq_host, "wo": wo_host, "maskt": mask})
    return in_maps


_nc_cache = {}


def get_nc(debug=False, **opts):
    key = (debug, tuple(sorted(opts.items())))
    if key not in _nc_cache:
        _nc_cache[key] = build(debug=debug, **opts)
    return _nc_cache[key]


def run(x, w_qkv, w_o, debug=False, **opts):
    nc = get_nc(debug=debug, **opts)
    in_maps = prep_inputs(x, w_qkv, w_o)
    res = bass_utils.run_bass_kernel_spmd(nc, in_maps, core_ids=list(range(NCORES)))
    return res


def kernel(x, w_qkv, w_o):
    res = run(x, w_qkv, w_o)
    y = res.results[0]["y"].astype(np.float64)
    for c in range(1, NCORES):
        y += res.results[c]["y"]
    return y.astype(np.float32).reshape(B, T, D)

